# revision 17
# baseline (speedup 1.0000x reference)
"""DualStreamEncoderAttention Trainium2 kernel (v3).

Sharding: 8 cores = 4 samples x 2 head-groups (8 heads each). Each core
computes, for its sample, both streams' QKV(+RoPE) for its 8 heads,
cross-stream attention, and a partial out-projection over its heads'
rows of Wout. The host sums the two partial projections per sample.

v3 speed strategy (S=1024, H=1024, D=64, 8 local heads):
  - PV reformulated with es as the matmul stationary ([128k x 128q]
    slices) and [v | ones] as the 65-wide moving operand: the PE pays 65
    columns per 128x128x65 block instead of 512 columns for 65 output
    rows. attn lands in [q, hd] orientation; per-query softmax
    denominators live on the partition axis, so normalization is a plain
    per-partition tensor_scalar multiply.
  - attn [q, hd] -> [hd, q] via 128x128 XBAR dma transposes (bf16), out
    projection is a bf16 single-accumulation pass per [128S x 512oc]
    tile, split into an early (heads 0-3) and late (heads 4-7) partial
    merged on DVE.
  - softmax exp is split across engines: 12 of 16 key-chunks per head
    run fp8-DoubleRow QK^T + exact exp on the Activation engine; 4 run
    bf16 QK^T (from bf16 rope copies) + a Schraudolph int16/bf16
    bit-pun exp on the Vector engine. The pun's bounded sawtooth error
    (+-4%) replaces the fp8 score error on those chunks, keeping overall
    relmax at baseline while cutting ACT exp work by 25%.
  - LayerNorm folded on host (xhat pre-transposed, gamma into W, beta
    via a per-block scalar add + host-side V-shift correction).
"""

import sys

for _p in ("/opt/trn_rl_repo", "/root/.axon_site/_ro/trn_rl_repo"):
    if _p not in sys.path:
        sys.path.insert(0, _p)

import numpy as np

S = 1024
H = 1024
NH = 16
D = 64
NHL = 8          # heads per core
P = 128
N_CORES = 8
LN_EPS = 1e-5
ROPE_BASE = 10000.0
SCALE = float(D) ** -0.5

# fast-exp (Schraudolph bf16 pun) constants: i16 = rint(A*score + B),
# bitcast bf16 ~= exp(score*SCALE) * 2^-sigma residual, sigma minimax.
LOG2E = 1.4426950408889634
FX_SIGMA = 0.0579
A_FX = SCALE * LOG2E * 128.0
B_FX = 16256.0 - 128.0 * FX_SIGMA
FX_TST = (0, 1)   # key-chunks tst<2 of each target stream use the pun path

_PROGRAM = None


def _rope_tables(height, width, head_dim=D):
    """Mirror of reference.rope_2d_tables in numpy float32."""
    height = int(height)
    width = int(width)
    dim_x = head_dim // 2
    dim_y = head_dim - dim_x
    inv_fx = 1.0 / (ROPE_BASE ** (np.arange(0, dim_x, 2, dtype=np.float32) / np.float32(dim_x)))
    inv_fy = 1.0 / (ROPE_BASE ** (np.arange(0, dim_y, 2, dtype=np.float32) / np.float32(dim_y)))
    fx = np.arange(width, dtype=np.float32)[:, None] * inv_fx[None, :]
    fy = np.arange(height, dtype=np.float32)[:, None] * inv_fy[None, :]
    fx = np.concatenate([fx, fx], axis=-1)  # [W, dim_x]
    fy = np.concatenate([fy, fy], axis=-1)  # [H, dim_y]
    cos = np.concatenate([
        np.broadcast_to(np.cos(fx)[None, :, :], (height, width, dim_x)),
        np.broadcast_to(np.cos(fy)[:, None, :], (height, width, dim_y)),
    ], axis=-1).reshape(height * width, head_dim).astype(np.float32)
    sin = np.concatenate([
        np.broadcast_to(np.sin(fx)[None, :, :], (height, width, dim_x)),
        np.broadcast_to(np.sin(fy)[:, None, :], (height, width, dim_y)),
    ], axis=-1).reshape(height * width, head_dim).astype(np.float32)
    return cos, sin


def _build_program(do_compile=True):
    import concourse.mybir as mybir
    import concourse.tile as tile
    from concourse import bacc

    f32 = mybir.dt.float32
    bf16 = mybir.dt.bfloat16
    fp8 = mybir.dt.float8e4
    i16 = mybir.dt.int16
    DR = mybir.MatmulPerfMode.DoubleRow
    AF = mybir.ActivationFunctionType
    ALU = mybir.AluOpType

    nc = bacc.Bacc("TRN2")

    # ---- DRAM parameters (per-core tensors; same program on all cores) ----
    xh_d = [nc.dram_tensor(f"xh_s{s}", [H, S], bf16, kind="ExternalInput") for s in range(2)]
    wqk_d = [nc.dram_tensor(f"wqk_s{s}", [H, 2 * NHL * D], bf16, kind="ExternalInput") for s in range(2)]
    wv_d = [nc.dram_tensor(f"wv_s{s}", [H, NHL * D], bf16, kind="ExternalInput") for s in range(2)]
    cqk_d = [nc.dram_tensor(f"cqk_s{s}", [P, 8], f32, kind="ExternalInput") for s in range(2)]
    wout_d = [nc.dram_tensor(f"wout_s{s}", [NHL * D, H], bf16, kind="ExternalInput") for s in range(2)]
    cos_d = nc.dram_tensor("cosdr", [P, 2 * S], bf16, kind="ExternalInput")
    sin_d = nc.dram_tensor("sindr", [P, 2 * S], bf16, kind="ExternalInput")  # pre-negated i=0 half
    out_d = [nc.dram_tensor(f"out_s{s}", [S, H], f32, kind="ExternalOutput") for s in range(2)]

    with tile.TileContext(nc) as tc:
        with (
            tc.tile_pool(name="consts", bufs=1) as consts,
            tc.tile_pool(name="persist", bufs=1) as persist,
            tc.tile_pool(name="esb", bufs=4) as esb_p,
            tc.tile_pool(name="small", bufs=2) as small,
            tc.tile_pool(name="mmp", bufs=2, space="PSUM") as mmp,
            tc.tile_pool(name="scp", bufs=2, space="PSUM") as scp,
            tc.tile_pool(name="accp", bufs=1, space="PSUM") as accp,
        ):
            cosdr = consts.tile([P, 2, S], bf16, tag="cosdr")
            sindr = consts.tile([P, 2, S], bf16, tag="sindr")
            with tc.high_priority(offset=-60):
                nc.sync.dma_start(out=cosdr, in_=cos_d[:].rearrange("p (i s) -> p i s", i=2))
                nc.sync.dma_start(out=sindr, in_=sin_d[:].rearrange("p (i s) -> p i s", i=2))
            zeroc = consts.tile([P, 1], f32, tag="zeroc")
            nc.vector.memset(zeroc, 0.0)
            # warm the PE p-state while the first DMAs land
            warm = consts.tile([P, 512], f32, tag="warm")
            nc.vector.memset(warm, 0.0)
            wps = mmp.tile([P, 512], f32, tag="mm", name="wps")
            for wi in range(4):
                nc.tensor.matmul(wps[0:1, :], zeroc, warm, start=(wi == 0), stop=(wi == 3))

            # persistent per-stream state
            qdr = [[persist.tile([P, 2, S], fp8, tag=f"qdr{s}_{ht}", name=f"qdr{s}_{ht}")
                    for ht in range(2)] for s in range(2)]
            kdr = [[persist.tile([P, 2, S], fp8, tag=f"kdr{s}_{ht}", name=f"kdr{s}_{ht}")
                    for ht in range(2)] for s in range(2)]
            # bf16 rope output, head-pair packed: pair tile [128 = (a,i)*32+dl, S]
            qbf = [[[persist.tile([P, S], bf16, tag=f"qbf{s}_{ht}_{pr}", name=f"qbf{s}_{ht}_{pr}")
                     for pr in range(2)] for ht in range(2)] for s in range(2)]
            kbf = [[[persist.tile([P, len(FX_TST) * P], bf16, tag=f"kbf{s}_{ht}_{pr}", name=f"kbf{s}_{ht}_{pr}")
                     for pr in range(2)] for ht in range(2)] for s in range(2)]
            v_sb = [[persist.tile([P, NHL, D + 1], bf16, tag=f"v{s}_{st}", name=f"v{s}_{st}")
                     for st in range(8)] for s in range(2)]
            attn2 = [[persist.tile([P, NHL * D, ], bf16, tag=f"at2_{s}_{qc}", name=f"at2_{s}_{qc}")
                      for qc in range(8)] for s in range(2)]
            attnT = [[persist.tile([P, S], bf16, tag=f"atT_{s}_{p}", name=f"atT_{s}_{p}")
                      for p in range(4)] for s in range(2)]

            # ---------------- prep helpers ----------------
            def load_stream(s, prep_p):
                xh = [prep_p.tile([P, S], bf16, tag=f"xh{s}_{hc}", name=f"xh{s}_{hc}") for hc in range(8)]
                qs = [nc.sync, nc.gpsimd, nc.scalar]
                for hc in range(8):
                    qs[hc % 3].dma_start(out=xh[hc], in_=xh_d[s][hc * P:(hc + 1) * P, :])
                cqk_sb = prep_p.tile([P, 8], f32, tag="cqk", bufs=2, name="cqk_sb")
                nc.sync.dma_start(out=cqk_sb, in_=cqk_d[s][:])
                return xh, cqk_sb

            def qk_group(s, xh, cqk_sb, ht, prep_p):
                """q+k projections, rope, fp8 DR tiles + bf16 side copies."""
                wqfs = {}
                for qk in range(2):
                    for half in range(2):
                        b = qk * 4 + ht * 2 + half
                        wqf = prep_p.tile([P, 8, P], bf16, tag="wqf", bufs=4, name="wqf")
                        (nc.sync if (qk + half) % 2 == 0 else nc.gpsimd).dma_start(
                            out=wqf,
                            in_=wqk_d[s][:, b * P:(b + 1) * P].rearrange("(c p) n -> p c n", p=P))
                        wqfs[(qk, half)] = wqf
                # bf16 rope staging (DR partition layout)
                qbfdr = prep_p.tile([P, 2, S], bf16, tag="qbfdr", bufs=2, name="qbfdr")
                kbfdr = prep_p.tile([P, 2, len(FX_TST) * P], bf16, tag="kbfdr", bufs=2, name="kbfdr")
                nfx = len(FX_TST) * P
                for sc in range(2):
                    csl = slice(sc * 512, (sc + 1) * 512)
                    for qk in range(2):
                        dst = (qdr if qk == 0 else kdr)[s]
                        stg_t = [None, None]
                        for half in range(2):
                            b = qk * 4 + ht * 2 + half
                            psq = mmp.tile([P, 512], f32, tag="mm", name="psq")
                            for kc in range(8):
                                nc.tensor.matmul(
                                    psq,
                                    wqfs[(qk, half)][:, kc, :],
                                    xh[kc][:, csl],
                                    start=(kc == 0), stop=(kc == 7),
                                )
                            stg = prep_p.tile([P, 512], bf16, tag="stg", bufs=5, name="stg")
                            nc.vector.tensor_scalar_add(stg, psq, cqk_sb[:, b:b + 1])
                            stg_t[half] = stg
                        for i in range(2):
                            tmp = small.tile([P, 512], bf16, tag="rtmp", bufs=3, name="rtmp")
                            nc.gpsimd.tensor_mul(tmp, stg_t[1 - i], sindr[:, i, csl])
                            qc = small.tile([P, 512], bf16, tag="rqc", bufs=3, name="rqc")
                            nc.gpsimd.tensor_mul(qc, stg_t[i], cosdr[:, i, csl])
                            nc.vector.tensor_add(dst[ht][:, i, csl], tmp, qc)
                            # bf16 side copies for the fast-exp score path
                            if qk == 0:
                                nc.gpsimd.tensor_add(qbfdr[:, i, csl], tmp, qc)
                            elif sc == 0:
                                nc.gpsimd.tensor_add(kbfdr[:, i, 0:nfx],
                                                     tmp[:, 0:nfx], qc[:, 0:nfx])
                # partition shuffle DR layout -> head-pair [64|64] layout
                for pr in range(2):
                    for a in range(2):
                        hh = 2 * pr + a
                        for i in range(2):
                            po = slice(64 * a + 32 * i, 64 * a + 32 * i + 32)
                            pi = slice(32 * hh, 32 * hh + 32)
                            nc.gpsimd.dma_start(out=qbf[s][ht][pr][po, :], in_=qbfdr[pi, i, :])
                            nc.sync.dma_start(out=kbf[s][ht][pr][po, :], in_=kbfdr[pi, i, :])

            def v_load(s, prep_p):
                wvf = prep_p.tile([P, 8, NHL * D], bf16, tag=f"wvf{s}", name="wvf")
                nc.gpsimd.dma_start(out=wvf, in_=wv_d[s][:].rearrange("(c p) n -> p c n", p=P))
                for st in range(8):
                    nc.gpsimd.memset(v_sb[s][st][:, :, D:D + 1], 1.0)
                return wvf

            def v_fills(s, xh, wvf, halves=(0, 1)):
                for nh in halves:
                    nsl = slice(nh * 256, (nh + 1) * 256)
                    for st in range(8):
                        psv = mmp.tile([P, 512], f32, tag="mm", name="psv")
                        for kc in range(8):
                            nc.tensor.matmul(
                                psv[:, 0:256],
                                xh[kc][:, st * P:(st + 1) * P],
                                wvf[:, kc, nsl],
                                start=(kc == 0), stop=(kc == 7),
                            )
                        nc.vector.tensor_copy(
                            out=v_sb[s][st][:, 4 * nh:4 * nh + 4, 0:D],
                            in_=psv[:, 0:256].rearrange("p (h d) -> p h d", d=D),
                        )

            # ---------------- attention per head (all 16 key-chunks) ---------
            # spread the 4 fast-exp chunks through the head; lead with an
            # ACT chunk so the PV accumulation group never waits on DVE
            TST_ORDER = (2, 0, 3, 4, 5, 1, 6, 7)

            def head_attn(s, ht, hh):
                h = 4 * ht + hh
                pr = slice(32 * hh, 32 * hh + 32)
                pair, a = hh // 2, hh % 2
                pp64 = slice(64 * a, 64 * a + 64)
                accs = [accp.tile([P, 4, D + 1], f32, tag="accA", name="accA"),
                        accp.tile([P, 4, D + 1], f32, tag="accB", name="accB")]
                for cidx, (ts, tst) in enumerate(
                        (t, o) for t in range(2) for o in TST_ORDER):
                    first, last = cidx == 0, cidx == 15
                    sc_ps = scp.tile([P, S], f32, tag="sc", name="sc_ps")
                    if tst in FX_TST:
                        # bf16 scores + vector-engine fast-exp pun
                        for sc in range(2):
                            csl = slice(sc * 512, (sc + 1) * 512)
                            nc.tensor.matmul(
                                sc_ps[:, csl],
                                kbf[ts][ht][pair][pp64, tst * P:(tst + 1) * P],
                                qbf[s][ht][pair][pp64, csl],
                            )
                        esx = esb_p.tile([P, S], i16, tag="esx", bufs=3, name="esx")
                        with tc.high_priority(offset=-40):
                            nc.vector.tensor_scalar(
                                out=esx, in0=sc_ps,
                                scalar1=A_FX, scalar2=B_FX,
                                op0=ALU.mult, op1=ALU.add,
                            )
                        es = esx.bitcast(bf16)
                    else:
                        for sc in range(2):
                            csl = slice(sc * 512, (sc + 1) * 512)
                            nc.tensor.matmul(
                                sc_ps[:, csl],
                                kdr[ts][ht][pr, :, tst * P:(tst + 1) * P],
                                qdr[s][ht][pr, :, csl],
                                perf_mode=DR,
                                tile_position=(32 * hh, 0),
                            )
                        est = esb_p.tile([P, S], bf16, tag="es", bufs=5, name="es")
                        nc.scalar.activation(out=est, in_=sc_ps, func=AF.Exp,
                                             bias=zeroc, scale=SCALE)
                        es = est
                    for qch in range(8):
                        nc.tensor.matmul(
                            accs[qch // 4][:, qch % 4, :],
                            es[:, qch * P:(qch + 1) * P],
                            v_sb[ts][tst][:, h, :],
                            start=first, stop=last,
                        )
                # normalize: per-q denominators sit on the free axis (col 64)
                for grp in range(2):
                    rr = small.tile([P, 4], f32, tag="rr", bufs=2, name="rr")
                    nc.vector.reciprocal(out=rr, in_=accs[grp][:, :, D])
                    for j in range(4):
                        qch = grp * 4 + j
                        nc.vector.tensor_scalar_mul(
                            attn2[s][qch][:, h * D:(h + 1) * D],
                            accs[grp][:, j, 0:D],
                            rr[:, j:j + 1],
                        )

            def transposes(s, ps):
                for p in ps:
                    for qch in range(8):
                        nc.sync.dma_start_transpose(
                            out=attnT[s][p][:, qch * P:(qch + 1) * P],
                            in_=attn2[s][qch][:, p * P:(p + 1) * P],
                        )

            # ---------------- out-projection ----------------
            opar = {}

            def tail_a(s, wop, wo_t):
                for p in range(2):
                    nc.sync.dma_start(out=wo_t[p], in_=wout_d[s][p * P:(p + 1) * P, :])
                for st in range(8):
                    for oc in range(2):
                        pso = mmp.tile([P, 512], f32, tag="mm", name="pso")
                        for p in range(2):
                            nc.tensor.matmul(
                                pso,
                                attnT[s][p][:, st * P:(st + 1) * P],
                                wo_t[p][:, oc * 512:(oc + 1) * 512],
                                start=(p == 0), stop=(p == 1),
                            )
                        op_t = wop.tile([P, 512], bf16, tag=f"opar{st}_{oc}", name="opar")
                        nc.vector.tensor_copy(out=op_t, in_=pso)
                        opar[(s, st, oc)] = op_t

            def tail_b(s, wop, wo_t):
                for p in range(2, 4):
                    nc.sync.dma_start(out=wo_t[p], in_=wout_d[s][p * P:(p + 1) * P, :])
                for st in range(8):
                    for oc in range(2):
                        pso = mmp.tile([P, 512], f32, tag="mm", name="pso")
                        for p in range(2, 4):
                            nc.tensor.matmul(
                                pso,
                                attnT[s][p][:, st * P:(st + 1) * P],
                                wo_t[p][:, oc * 512:(oc + 1) * 512],
                                start=(p == 2), stop=(p == 3),
                            )
                        osb = small.tile([P, 512], f32, tag="osb", bufs=3, name="osb")
                        nc.vector.tensor_add(osb, pso, opar[(s, st, oc)])
                        (nc.gpsimd if (st + oc) % 2 == 0 else nc.sync).dma_start(
                            out=out_d[s][st * P:(st + 1) * P, oc * 512:(oc + 1) * 512], in_=osb)

            # ---------------- emission ----------------
            with tc.tile_pool(name="prep", bufs=1) as prep_p:
                xh0, cq0 = load_stream(0, prep_p)
                qk_group(0, xh0, cq0, 0, prep_p)
                wvf0 = v_load(0, prep_p)
                with tc.high_priority(offset=-50):
                    v_fills(0, xh0, wvf0, halves=(0,))
                xh1, cq1 = load_stream(1, prep_p)
                qk_group(1, xh1, cq1, 0, prep_p)
                wvf1 = v_load(1, prep_p)
                v_fills(1, xh1, wvf1, halves=(0,))
                for hh in range(4):
                    head_attn(0, 0, hh)
                qk_group(0, xh0, cq0, 1, prep_p)
                v_fills(0, xh0, wvf0, halves=(1,))
                qk_group(1, xh1, cq1, 1, prep_p)
                v_fills(1, xh1, wvf1, halves=(1,))
                for hh in range(4):
                    head_attn(1, 0, hh)
                for hh in range(4):
                    head_attn(0, 1, hh)
            with tc.tile_pool(name="wo", bufs=1) as wop:
                wo_t0 = [wop.tile([P, H], bf16, tag=f"wo{p}", name=f"wo{p}") for p in range(4)]
                wo_t1 = [wop.tile([P, H], bf16, tag=f"wo{p}", name=f"wo{p}") for p in range(4)]
                transposes(0, (0, 1, 2, 3))
                tail_a(0, wop, wo_t0)
                tail_b(0, wop, wo_t0)
                transposes(1, (0, 1))
                tail_a(1, wop, wo_t1)
                for hh in range(4):
                    head_attn(1, 1, hh)
                transposes(1, (2, 3))
                tail_b(1, wop, wo_t1)

    if do_compile:
        nc.compile()
    return nc


def _host_prep(x_a, x_b, Wqkv_a, Wqkv_b, Wout_a, Wout_b,
               gamma_a, beta_a, gamma_b, beta_b, height, width):
    """Build the 8 per-core input maps."""
    import ml_dtypes
    cos, sin = _rope_tables(height, width)      # [S, 64]

    # DR rope tables [128, 2, S]: partition p = 32*hh + dl (repeats over hh)
    dl = np.arange(32)
    cos_dr = np.empty((P, 2, S), np.float32)
    sin_dr = np.empty((P, 2, S), np.float32)
    for hh in range(4):
        rows = 32 * hh + dl
        cos_dr[rows, 0, :] = cos[:, dl].T          # cos[s, dl]
        cos_dr[rows, 1, :] = cos[:, 32 + dl].T
        sin_dr[rows, 0, :] = -sin[:, dl].T         # pre-negated for i=0
        sin_dr[rows, 1, :] = sin[:, 32 + dl].T
    cos_dr = np.ascontiguousarray(cos_dr.reshape(P, 2 * S).astype(ml_dtypes.bfloat16))
    sin_dr = np.ascontiguousarray(sin_dr.reshape(P, 2 * S).astype(ml_dtypes.bfloat16))

    # host LayerNorm fold: xhat = r*(x-mu), shipped transposed [H, S]
    def xhat(x):
        x = x.astype(np.float32)
        mu = x.mean(axis=-1, keepdims=True)
        var = ((x - mu) ** 2).mean(axis=-1, keepdims=True)
        r = 1.0 / np.sqrt(var + LN_EPS)
        return ((x - mu) * r).astype(np.float32)

    streams = []
    vshifts = []
    for (W, Wo, g, b) in ((Wqkv_a, Wout_a, gamma_a, beta_a), (Wqkv_b, Wout_b, gamma_b, beta_b)):
        Wg = (W * g[:, None]).astype(np.float32)       # gamma-folded
        cfull = (b.astype(np.float64) @ W.astype(np.float64)).astype(np.float32)  # beta@W [3H]
        W4 = Wg.reshape(H, 3, NH, D)
        c4 = cfull.reshape(3, NH, D)
        per_hg = []
        for hg in range(2):
            h0 = hg * NHL
            # DR column order: blocks (qk, ht, half) of 128 cols = (hh, dl)
            cols = []
            ccols = []
            for qk in range(2):
                for ht in range(2):
                    for half in range(2):
                        for hh in range(4):
                            head = h0 + 4 * ht + hh
                            dsl = slice(32 * half, 32 * half + 32)
                            cols.append(W4[:, qk, head, dsl])      # [H, 32]
                            ccols.append(c4[qk, head, dsl])        # [32]
            wqk = np.ascontiguousarray(np.concatenate(cols, axis=1).astype(ml_dtypes.bfloat16))
            # beta@W per qk column as per-partition scalars [128, 8 blocks]
            cqk_blk = np.ascontiguousarray(
                np.concatenate(ccols).reshape(8, P).T.astype(np.float32))   # [128, 8]
            wv = np.ascontiguousarray(W4[:, 2, h0:h0 + NHL, :].reshape(H, NHL * D).astype(ml_dtypes.bfloat16))
            wout = np.ascontiguousarray(
                Wo.reshape(NH, D, H)[h0:h0 + NHL].reshape(NHL * D, H).astype(ml_dtypes.bfloat16))
            per_hg.append(dict(wqk=wqk, wv=wv, cqk=cqk_blk, wout=wout))
        # exact host-side V correction: beta@Wv shifts attn uniformly
        # (softmax weights sum to 1), so it lands as a constant row on out
        vshift = (cfull[2 * H:3 * H].astype(np.float64) @ Wo.astype(np.float64)).astype(np.float32)
        streams.append(per_hg)
        vshifts.append(vshift)

    in_maps = []
    B = x_a.shape[0]
    xh_a = [np.ascontiguousarray(xhat(x_a[b_i]).T.astype(ml_dtypes.bfloat16)) for b_i in range(B)]
    xh_b = [np.ascontiguousarray(xhat(x_b[b_i]).T.astype(ml_dtypes.bfloat16)) for b_i in range(B)]
    for c in range(N_CORES):
        b_i, hg = (c // 2) % B, c % 2
        m = {
            "xh_s0": xh_a[b_i],
            "xh_s1": xh_b[b_i],
            "cosdr": cos_dr, "sindr": sin_dr,
        }
        for s in range(2):
            blk = streams[s][hg]
            m[f"wqk_s{s}"] = blk["wqk"]
            m[f"wv_s{s}"] = blk["wv"]
            m[f"cqk_s{s}"] = blk["cqk"]
            m[f"wout_s{s}"] = blk["wout"]
        in_maps.append(m)
    return in_maps, vshifts


def kernel(x_a, x_b, Wqkv_a, Wqkv_b, Wout_a, Wout_b,
           gamma_a, beta_a, gamma_b, beta_b, height, width):
    from concourse.bass_utils import run_bass_kernel_spmd

    x_a = np.asarray(x_a, dtype=np.float32)
    x_b = np.asarray(x_b, dtype=np.float32)
    B = x_a.shape[0]
    in_maps, vshifts = _host_prep(x_a, x_b,
                         np.asarray(Wqkv_a, np.float32), np.asarray(Wqkv_b, np.float32),
                         np.asarray(Wout_a, np.float32), np.asarray(Wout_b, np.float32),
                         np.asarray(gamma_a, np.float32), np.asarray(beta_a, np.float32),
                         np.asarray(gamma_b, np.float32), np.asarray(beta_b, np.float32),
                         height, width)
    nc = _get_program()
    res = run_bass_kernel_spmd(nc, in_maps, list(range(N_CORES))).results
    out_a = np.empty((B, S, H), np.float32)
    out_b = np.empty((B, S, H), np.float32)
    for b_i in range(B):
        out_a[b_i] = res[2 * b_i]["out_s0"] + res[2 * b_i + 1]["out_s0"] + vshifts[0]
        out_b[b_i] = res[2 * b_i]["out_s1"] + res[2 * b_i + 1]["out_s1"] + vshifts[1]
    return out_a, out_b


def _get_program():
    global _PROGRAM
    if _PROGRAM is None:
        _PROGRAM = _build_program()
    return _PROGRAM


# revision 20
# speedup vs baseline: 1.0342x; 1.0342x over previous
"""DualStreamEncoderAttention Trainium2 kernel (v3).

Sharding: 8 cores = 4 samples x 2 head-groups (8 heads each). Each core
computes, for its sample, both streams' QKV(+RoPE) for its 8 heads,
cross-stream attention, and a partial out-projection over its heads'
rows of Wout. The host sums the two partial projections per sample.

v3 speed strategy (S=1024, H=1024, D=64, 8 local heads):
  - PV reformulated with es as the matmul stationary ([128k x 128q]
    slices) and [v | ones] as the 65-wide moving operand: the PE pays 65
    columns per 128x128x65 block instead of 512 columns for 65 output
    rows. attn lands in [q, hd] orientation; per-query softmax
    denominators live on the partition axis, so normalization is a plain
    per-partition tensor_scalar multiply.
  - attn [q, hd] -> [hd, q] via 128x128 XBAR dma transposes (bf16), out
    projection is a bf16 single-accumulation pass per [128S x 512oc]
    tile, split into an early (heads 0-3) and late (heads 4-7) partial
    merged on DVE.
  - softmax exp is split across engines: 12 of 16 key-chunks per head
    run fp8-DoubleRow QK^T + exact exp on the Activation engine; 4 run
    bf16 QK^T (from bf16 rope copies) + a Schraudolph int16/bf16
    bit-pun exp on the Vector engine. The pun's bounded sawtooth error
    (+-4%) replaces the fp8 score error on those chunks, keeping overall
    relmax at baseline while cutting ACT exp work by 25%.
  - LayerNorm folded on host (xhat pre-transposed, gamma into W, beta
    via a per-block scalar add + host-side V-shift correction).
"""

import sys

for _p in ("/opt/trn_rl_repo", "/root/.axon_site/_ro/trn_rl_repo"):
    if _p not in sys.path:
        sys.path.insert(0, _p)

import numpy as np

S = 1024
H = 1024
NH = 16
D = 64
NHL = 8          # heads per core
P = 128
N_CORES = 8
LN_EPS = 1e-5
ROPE_BASE = 10000.0
SCALE = float(D) ** -0.5

# fast-exp (Schraudolph bf16 pun) constants: i16 = rint(A*score + B),
# bitcast bf16 ~= exp(score*SCALE) * 2^-sigma residual, sigma minimax.
LOG2E = 1.4426950408889634
FX_SIGMA = 0.0579
A_FX = SCALE * LOG2E * 128.0
B_FX = 16256.0 - 128.0 * FX_SIGMA
FX_TST = (0, 1)   # key-chunks tst<2 of each target stream use the pun path

_PROGRAM = None


def _rope_tables(height, width, head_dim=D):
    """Mirror of reference.rope_2d_tables in numpy float32."""
    height = int(height)
    width = int(width)
    dim_x = head_dim // 2
    dim_y = head_dim - dim_x
    inv_fx = 1.0 / (ROPE_BASE ** (np.arange(0, dim_x, 2, dtype=np.float32) / np.float32(dim_x)))
    inv_fy = 1.0 / (ROPE_BASE ** (np.arange(0, dim_y, 2, dtype=np.float32) / np.float32(dim_y)))
    fx = np.arange(width, dtype=np.float32)[:, None] * inv_fx[None, :]
    fy = np.arange(height, dtype=np.float32)[:, None] * inv_fy[None, :]
    fx = np.concatenate([fx, fx], axis=-1)  # [W, dim_x]
    fy = np.concatenate([fy, fy], axis=-1)  # [H, dim_y]
    cos = np.concatenate([
        np.broadcast_to(np.cos(fx)[None, :, :], (height, width, dim_x)),
        np.broadcast_to(np.cos(fy)[:, None, :], (height, width, dim_y)),
    ], axis=-1).reshape(height * width, head_dim).astype(np.float32)
    sin = np.concatenate([
        np.broadcast_to(np.sin(fx)[None, :, :], (height, width, dim_x)),
        np.broadcast_to(np.sin(fy)[:, None, :], (height, width, dim_y)),
    ], axis=-1).reshape(height * width, head_dim).astype(np.float32)
    return cos, sin


def _build_program(do_compile=True):
    import concourse.mybir as mybir
    import concourse.tile as tile
    from concourse import bacc

    f32 = mybir.dt.float32
    bf16 = mybir.dt.bfloat16
    fp8 = mybir.dt.float8e4
    i16 = mybir.dt.int16
    DR = mybir.MatmulPerfMode.DoubleRow
    AF = mybir.ActivationFunctionType
    ALU = mybir.AluOpType

    nc = bacc.Bacc("TRN2")

    # ---- DRAM parameters (per-core tensors; same program on all cores) ----
    xh_d = [nc.dram_tensor(f"xh_s{s}", [H, S], bf16, kind="ExternalInput") for s in range(2)]
    wqk_d = [nc.dram_tensor(f"wqk_s{s}", [H, 2 * NHL * D], bf16, kind="ExternalInput") for s in range(2)]
    wv_d = [nc.dram_tensor(f"wv_s{s}", [H, NHL * D], bf16, kind="ExternalInput") for s in range(2)]
    cqk_d = [nc.dram_tensor(f"cqk_s{s}", [P, 8], f32, kind="ExternalInput") for s in range(2)]
    wout_d = [nc.dram_tensor(f"wout_s{s}", [NHL * D, H], bf16, kind="ExternalInput") for s in range(2)]
    cos_d = nc.dram_tensor("cosdr", [P, 2 * S], bf16, kind="ExternalInput")
    sin_d = nc.dram_tensor("sindr", [P, 2 * S], bf16, kind="ExternalInput")  # pre-negated i=0 half
    out_d = [nc.dram_tensor(f"out_s{s}", [S, H], f32, kind="ExternalOutput") for s in range(2)]

    with tile.TileContext(nc) as tc:
        with (
            tc.tile_pool(name="consts", bufs=1) as consts,
            tc.tile_pool(name="persist", bufs=1) as persist,
            tc.tile_pool(name="esb", bufs=4) as esb_p,
            tc.tile_pool(name="small", bufs=2) as small,
            tc.tile_pool(name="mmp", bufs=2, space="PSUM") as mmp,
            tc.tile_pool(name="scp", bufs=2, space="PSUM") as scp,
            tc.tile_pool(name="accp", bufs=1, space="PSUM") as accp,
        ):
            cosdr = consts.tile([P, 2, S], bf16, tag="cosdr")
            sindr = consts.tile([P, 2, S], bf16, tag="sindr")
            with tc.high_priority(offset=-60):
                nc.sync.dma_start(out=cosdr, in_=cos_d[:].rearrange("p (i s) -> p i s", i=2))
                nc.sync.dma_start(out=sindr, in_=sin_d[:].rearrange("p (i s) -> p i s", i=2))
            zeroc = consts.tile([P, 1], f32, tag="zeroc")
            nc.vector.memset(zeroc, 0.0)
            # warm the PE p-state while the first DMAs land
            warm = consts.tile([P, 512], f32, tag="warm")
            nc.vector.memset(warm, 0.0)
            wps = mmp.tile([P, 512], f32, tag="mm", name="wps")
            for wi in range(4):
                nc.tensor.matmul(wps[0:1, :], zeroc, warm, start=(wi == 0), stop=(wi == 3))

            # persistent per-stream state
            qdr = [[persist.tile([P, 2, S], fp8, tag=f"qdr{s}_{ht}", name=f"qdr{s}_{ht}")
                    for ht in range(2)] for s in range(2)]
            kdr = [[persist.tile([P, 2, S], fp8, tag=f"kdr{s}_{ht}", name=f"kdr{s}_{ht}")
                    for ht in range(2)] for s in range(2)]
            # bf16 rope output, head-pair packed: pair tile [128 = (a,i)*32+dl, S]
            qbf = [[[persist.tile([P, S], bf16, tag=f"qbf{s}_{ht}_{pr}", name=f"qbf{s}_{ht}_{pr}")
                     for pr in range(2)] for ht in range(2)] for s in range(2)]
            kbf = [[[persist.tile([P, len(FX_TST) * P], bf16, tag=f"kbf{s}_{ht}_{pr}", name=f"kbf{s}_{ht}_{pr}")
                     for pr in range(2)] for ht in range(2)] for s in range(2)]
            v_sb = [[persist.tile([P, NHL, D + 1], bf16, tag=f"v{s}_{st}", name=f"v{s}_{st}")
                     for st in range(8)] for s in range(2)]
            attn2 = [[persist.tile([P, NHL * D, ], bf16, tag=f"at2_{s}_{qc}", name=f"at2_{s}_{qc}")
                      for qc in range(8)] for s in range(2)]
            attnT = [[persist.tile([P, S], bf16, tag=f"atT_{s}_{p}", name=f"atT_{s}_{p}")
                      for p in range(4)] for s in range(2)]

            # ---------------- prep helpers ----------------
            def load_stream(s, prep_p):
                xh = [prep_p.tile([P, S], bf16, tag=f"xh{s}_{hc}", name=f"xh{s}_{hc}") for hc in range(8)]
                qs = [nc.sync, nc.gpsimd, nc.scalar]
                for hc in range(8):
                    qs[hc % 3].dma_start(out=xh[hc], in_=xh_d[s][hc * P:(hc + 1) * P, :])
                cqk_sb = prep_p.tile([P, 8], f32, tag="cqk", bufs=2, name="cqk_sb")
                nc.sync.dma_start(out=cqk_sb, in_=cqk_d[s][:])
                return xh, cqk_sb

            def qk_group(s, xh, cqk_sb, ht, prep_p):
                """q+k projections, rope, fp8 DR tiles + bf16 side copies."""
                wqfs = {}
                for qk in range(2):
                    for half in range(2):
                        b = qk * 4 + ht * 2 + half
                        wqf = prep_p.tile([P, 8, P], bf16, tag="wqf", bufs=4, name="wqf")
                        (nc.sync if (qk + half) % 2 == 0 else nc.gpsimd).dma_start(
                            out=wqf,
                            in_=wqk_d[s][:, b * P:(b + 1) * P].rearrange("(c p) n -> p c n", p=P))
                        wqfs[(qk, half)] = wqf
                # bf16 rope staging (DR partition layout)
                qbfdr = prep_p.tile([P, 2, S], bf16, tag="qbfdr", bufs=2, name="qbfdr")
                kbfdr = prep_p.tile([P, 2, len(FX_TST) * P], bf16, tag="kbfdr", bufs=2, name="kbfdr")
                nfx = len(FX_TST) * P
                for sc in range(2):
                    csl = slice(sc * 512, (sc + 1) * 512)
                    for qk in range(2):
                        dst = (qdr if qk == 0 else kdr)[s]
                        stg_t = [None, None]
                        for half in range(2):
                            b = qk * 4 + ht * 2 + half
                            psq = mmp.tile([P, 512], f32, tag="mm", name="psq")
                            for kc in range(8):
                                nc.tensor.matmul(
                                    psq,
                                    wqfs[(qk, half)][:, kc, :],
                                    xh[kc][:, csl],
                                    start=(kc == 0), stop=(kc == 7),
                                )
                            stg = prep_p.tile([P, 512], bf16, tag="stg", bufs=5, name="stg")
                            nc.vector.tensor_scalar_add(stg, psq, cqk_sb[:, b:b + 1])
                            stg_t[half] = stg
                        for i in range(2):
                            tmp = small.tile([P, 512], bf16, tag="rtmp", bufs=3, name="rtmp")
                            nc.gpsimd.tensor_mul(tmp, stg_t[1 - i], sindr[:, i, csl])
                            qc = small.tile([P, 512], bf16, tag="rqc", bufs=3, name="rqc")
                            nc.gpsimd.tensor_mul(qc, stg_t[i], cosdr[:, i, csl])
                            nc.vector.tensor_add(dst[ht][:, i, csl], tmp, qc)
                            # bf16 side copies for the fast-exp score path
                            if qk == 0:
                                nc.gpsimd.tensor_add(qbfdr[:, i, csl], tmp, qc)
                            elif sc == 0:
                                nc.gpsimd.tensor_add(kbfdr[:, i, 0:nfx],
                                                     tmp[:, 0:nfx], qc[:, 0:nfx])
                # partition shuffle DR layout -> head-pair [64|64] layout
                for pr in range(2):
                    for a in range(2):
                        hh = 2 * pr + a
                        for i in range(2):
                            po = slice(64 * a + 32 * i, 64 * a + 32 * i + 32)
                            pi = slice(32 * hh, 32 * hh + 32)
                            nc.gpsimd.dma_start(out=qbf[s][ht][pr][po, :], in_=qbfdr[pi, i, :])
                            nc.sync.dma_start(out=kbf[s][ht][pr][po, :], in_=kbfdr[pi, i, :])

            def v_load(s, prep_p):
                wvf = prep_p.tile([P, 8, NHL * D], bf16, tag=f"wvf{s}", name="wvf")
                nc.gpsimd.dma_start(out=wvf, in_=wv_d[s][:].rearrange("(c p) n -> p c n", p=P))
                for st in range(8):
                    nc.gpsimd.memset(v_sb[s][st][:, :, D:D + 1], 1.0)
                return wvf

            def v_fills(s, xh, wvf, halves=(0, 1)):
                for nh in halves:
                    nsl = slice(nh * 256, (nh + 1) * 256)
                    for st in range(8):
                        psv = mmp.tile([P, 512], f32, tag="mm", name="psv")
                        for kc in range(8):
                            nc.tensor.matmul(
                                psv[:, 0:256],
                                xh[kc][:, st * P:(st + 1) * P],
                                wvf[:, kc, nsl],
                                start=(kc == 0), stop=(kc == 7),
                            )
                        nc.vector.tensor_copy(
                            out=v_sb[s][st][:, 4 * nh:4 * nh + 4, 0:D],
                            in_=psv[:, 0:256].rearrange("p (h d) -> p h d", d=D),
                        )

            # ---------------- attention per head (all 16 key-chunks) ---------
            # spread the 4 fast-exp chunks through the head; lead with an
            # ACT chunk so the PV accumulation group never waits on DVE
            TST_ORDER = (2, 0, 3, 4, 5, 1, 6, 7)

            acc_live = {}

            def head_attn(s, ht, hh, ts_range=(0, 1)):
                h = 4 * ht + hh
                pr = slice(32 * hh, 32 * hh + 32)
                pair, a = hh // 2, hh % 2
                pp64 = slice(64 * a, 64 * a + 64)
                if 0 in ts_range:
                    accs = [accp.tile([P, 4, D + 1], f32, tag="accA", name="accA"),
                            accp.tile([P, 4, D + 1], f32, tag="accB", name="accB")]
                    acc_live[(s, h)] = accs
                else:
                    accs = acc_live[(s, h)]
                for cidx, (ts, tst) in enumerate(
                        (t, o) for t in range(2) for o in TST_ORDER):
                    if ts not in ts_range:
                        continue
                    first, last = cidx == 0, cidx == 15
                    sc_ps = scp.tile([P, S], f32, tag="sc", name="sc_ps")
                    if tst in FX_TST:
                        # bf16 scores + vector-engine fast-exp pun
                        for sc in range(2):
                            csl = slice(sc * 512, (sc + 1) * 512)
                            nc.tensor.matmul(
                                sc_ps[:, csl],
                                kbf[ts][ht][pair][pp64, tst * P:(tst + 1) * P],
                                qbf[s][ht][pair][pp64, csl],
                            )
                        esx = esb_p.tile([P, S], i16, tag="esx", bufs=3, name="esx")
                        with tc.high_priority(offset=-40):
                            nc.vector.tensor_scalar(
                                out=esx, in0=sc_ps,
                                scalar1=A_FX, scalar2=B_FX,
                                op0=ALU.mult, op1=ALU.add,
                            )
                        es = esx.bitcast(bf16)
                    else:
                        for sc in range(2):
                            csl = slice(sc * 512, (sc + 1) * 512)
                            nc.tensor.matmul(
                                sc_ps[:, csl],
                                kdr[ts][ht][pr, :, tst * P:(tst + 1) * P],
                                qdr[s][ht][pr, :, csl],
                                perf_mode=DR,
                                tile_position=(32 * hh, 0),
                            )
                        est = esb_p.tile([P, S], bf16, tag="es", bufs=5, name="es")
                        nc.scalar.activation(out=est, in_=sc_ps, func=AF.Exp,
                                             bias=zeroc, scale=SCALE)
                        es = est
                    for qch in range(8):
                        nc.tensor.matmul(
                            accs[qch // 4][:, qch % 4, :],
                            es[:, qch * P:(qch + 1) * P],
                            v_sb[ts][tst][:, h, :],
                            start=first, stop=last,
                        )
                if 1 not in ts_range:
                    return
                # normalize: per-q denominators sit on the free axis (col 64)
                for grp in range(2):
                    rr = small.tile([P, 4], f32, tag="rr", bufs=2, name="rr")
                    nc.vector.reciprocal(out=rr, in_=accs[grp][:, :, D])
                    for j in range(4):
                        qch = grp * 4 + j
                        nc.vector.tensor_scalar_mul(
                            attn2[s][qch][:, h * D:(h + 1) * D],
                            accs[grp][:, j, 0:D],
                            rr[:, j:j + 1],
                        )

            def transposes(s, ps):
                for p in ps:
                    for qch in range(8):
                        nc.sync.dma_start_transpose(
                            out=attnT[s][p][:, qch * P:(qch + 1) * P],
                            in_=attn2[s][qch][:, p * P:(p + 1) * P],
                        )

            # ---------------- out-projection ----------------
            opar = {}

            def tail_a(s, wop, wo_t):
                for p in range(2):
                    nc.sync.dma_start(out=wo_t[p], in_=wout_d[s][p * P:(p + 1) * P, :])
                for st in range(8):
                    for oc in range(2):
                        pso = mmp.tile([P, 512], f32, tag="mm", name="pso")
                        for p in range(2):
                            nc.tensor.matmul(
                                pso,
                                attnT[s][p][:, st * P:(st + 1) * P],
                                wo_t[p][:, oc * 512:(oc + 1) * 512],
                                start=(p == 0), stop=(p == 1),
                            )
                        op_t = wop.tile([P, 512], bf16, tag=f"opar{st}_{oc}", name="opar")
                        nc.vector.tensor_copy(out=op_t, in_=pso)
                        opar[(s, st, oc)] = op_t

            def tail_b(s, wop, wo_t):
                for p in range(2, 4):
                    nc.sync.dma_start(out=wo_t[p], in_=wout_d[s][p * P:(p + 1) * P, :])
                for st in range(8):
                    for oc in range(2):
                        pso = mmp.tile([P, 512], f32, tag="mm", name="pso")
                        for p in range(2, 4):
                            nc.tensor.matmul(
                                pso,
                                attnT[s][p][:, st * P:(st + 1) * P],
                                wo_t[p][:, oc * 512:(oc + 1) * 512],
                                start=(p == 2), stop=(p == 3),
                            )
                        osb = small.tile([P, 512], f32, tag="osb", bufs=3, name="osb")
                        nc.vector.tensor_add(osb, pso, opar[(s, st, oc)])
                        (nc.gpsimd if (st + oc) % 2 == 0 else nc.sync).dma_start(
                            out=out_d[s][st * P:(st + 1) * P, oc * 512:(oc + 1) * 512], in_=osb)

            # ---------------- emission ----------------
            with tc.tile_pool(name="prep", bufs=1) as prep_p:
                xh0, cq0 = load_stream(0, prep_p)
                qk_group(0, xh0, cq0, 0, prep_p)
                wvf0 = v_load(0, prep_p)
                with tc.high_priority(offset=-50):
                    v_fills(0, xh0, wvf0, halves=(0,))
                xh1, cq1 = load_stream(1, prep_p)
                head_attn(0, 0, 0, ts_range=(0,))
                qk_group(1, xh1, cq1, 0, prep_p)
                wvf1 = v_load(1, prep_p)
                v_fills(1, xh1, wvf1, halves=(0,))
                head_attn(0, 0, 0, ts_range=(1,))
                for hh in range(1, 4):
                    head_attn(0, 0, hh)
                qk_group(0, xh0, cq0, 1, prep_p)
                v_fills(0, xh0, wvf0, halves=(1,))
                for hh in range(4):
                    head_attn(1, 0, hh)
                qk_group(1, xh1, cq1, 1, prep_p)
                v_fills(1, xh1, wvf1, halves=(1,))
                for hh in range(4):
                    head_attn(0, 1, hh)
            with tc.tile_pool(name="wo", bufs=1) as wop:
                wo_t0 = [wop.tile([P, H], bf16, tag=f"wo{p}", name=f"wo{p}") for p in range(4)]
                wo_t1 = [wop.tile([P, H], bf16, tag=f"wo{p}", name=f"wo{p}") for p in range(4)]
                transposes(0, (0, 1, 2, 3))
                tail_a(0, wop, wo_t0)
                tail_b(0, wop, wo_t0)
                transposes(1, (0, 1))
                tail_a(1, wop, wo_t1)
                for hh in range(4):
                    head_attn(1, 1, hh)
                transposes(1, (2, 3))
                tail_b(1, wop, wo_t1)

    if do_compile:
        nc.compile()
    return nc


def _host_prep(x_a, x_b, Wqkv_a, Wqkv_b, Wout_a, Wout_b,
               gamma_a, beta_a, gamma_b, beta_b, height, width):
    """Build the 8 per-core input maps."""
    import ml_dtypes
    cos, sin = _rope_tables(height, width)      # [S, 64]

    # DR rope tables [128, 2, S]: partition p = 32*hh + dl (repeats over hh)
    dl = np.arange(32)
    cos_dr = np.empty((P, 2, S), np.float32)
    sin_dr = np.empty((P, 2, S), np.float32)
    for hh in range(4):
        rows = 32 * hh + dl
        cos_dr[rows, 0, :] = cos[:, dl].T          # cos[s, dl]
        cos_dr[rows, 1, :] = cos[:, 32 + dl].T
        sin_dr[rows, 0, :] = -sin[:, dl].T         # pre-negated for i=0
        sin_dr[rows, 1, :] = sin[:, 32 + dl].T
    cos_dr = np.ascontiguousarray(cos_dr.reshape(P, 2 * S).astype(ml_dtypes.bfloat16))
    sin_dr = np.ascontiguousarray(sin_dr.reshape(P, 2 * S).astype(ml_dtypes.bfloat16))

    # host LayerNorm fold: xhat = r*(x-mu), shipped transposed [H, S]
    def xhat(x):
        x = x.astype(np.float32)
        mu = x.mean(axis=-1, keepdims=True)
        var = ((x - mu) ** 2).mean(axis=-1, keepdims=True)
        r = 1.0 / np.sqrt(var + LN_EPS)
        return ((x - mu) * r).astype(np.float32)

    streams = []
    vshifts = []
    for (W, Wo, g, b) in ((Wqkv_a, Wout_a, gamma_a, beta_a), (Wqkv_b, Wout_b, gamma_b, beta_b)):
        Wg = (W * g[:, None]).astype(np.float32)       # gamma-folded
        cfull = (b.astype(np.float64) @ W.astype(np.float64)).astype(np.float32)  # beta@W [3H]
        W4 = Wg.reshape(H, 3, NH, D)
        c4 = cfull.reshape(3, NH, D)
        per_hg = []
        for hg in range(2):
            h0 = hg * NHL
            # DR column order: blocks (qk, ht, half) of 128 cols = (hh, dl)
            cols = []
            ccols = []
            for qk in range(2):
                for ht in range(2):
                    for half in range(2):
                        for hh in range(4):
                            head = h0 + 4 * ht + hh
                            dsl = slice(32 * half, 32 * half + 32)
                            cols.append(W4[:, qk, head, dsl])      # [H, 32]
                            ccols.append(c4[qk, head, dsl])        # [32]
            wqk = np.ascontiguousarray(np.concatenate(cols, axis=1).astype(ml_dtypes.bfloat16))
            # beta@W per qk column as per-partition scalars [128, 8 blocks]
            cqk_blk = np.ascontiguousarray(
                np.concatenate(ccols).reshape(8, P).T.astype(np.float32))   # [128, 8]
            wv = np.ascontiguousarray(W4[:, 2, h0:h0 + NHL, :].reshape(H, NHL * D).astype(ml_dtypes.bfloat16))
            wout = np.ascontiguousarray(
                Wo.reshape(NH, D, H)[h0:h0 + NHL].reshape(NHL * D, H).astype(ml_dtypes.bfloat16))
            per_hg.append(dict(wqk=wqk, wv=wv, cqk=cqk_blk, wout=wout))
        # exact host-side V correction: beta@Wv shifts attn uniformly
        # (softmax weights sum to 1), so it lands as a constant row on out
        vshift = (cfull[2 * H:3 * H].astype(np.float64) @ Wo.astype(np.float64)).astype(np.float32)
        streams.append(per_hg)
        vshifts.append(vshift)

    in_maps = []
    B = x_a.shape[0]
    xh_a = [np.ascontiguousarray(xhat(x_a[b_i]).T.astype(ml_dtypes.bfloat16)) for b_i in range(B)]
    xh_b = [np.ascontiguousarray(xhat(x_b[b_i]).T.astype(ml_dtypes.bfloat16)) for b_i in range(B)]
    for c in range(N_CORES):
        b_i, hg = (c // 2) % B, c % 2
        m = {
            "xh_s0": xh_a[b_i],
            "xh_s1": xh_b[b_i],
            "cosdr": cos_dr, "sindr": sin_dr,
        }
        for s in range(2):
            blk = streams[s][hg]
            m[f"wqk_s{s}"] = blk["wqk"]
            m[f"wv_s{s}"] = blk["wv"]
            m[f"cqk_s{s}"] = blk["cqk"]
            m[f"wout_s{s}"] = blk["wout"]
        in_maps.append(m)
    return in_maps, vshifts


def kernel(x_a, x_b, Wqkv_a, Wqkv_b, Wout_a, Wout_b,
           gamma_a, beta_a, gamma_b, beta_b, height, width):
    from concourse.bass_utils import run_bass_kernel_spmd

    x_a = np.asarray(x_a, dtype=np.float32)
    x_b = np.asarray(x_b, dtype=np.float32)
    B = x_a.shape[0]
    in_maps, vshifts = _host_prep(x_a, x_b,
                         np.asarray(Wqkv_a, np.float32), np.asarray(Wqkv_b, np.float32),
                         np.asarray(Wout_a, np.float32), np.asarray(Wout_b, np.float32),
                         np.asarray(gamma_a, np.float32), np.asarray(beta_a, np.float32),
                         np.asarray(gamma_b, np.float32), np.asarray(beta_b, np.float32),
                         height, width)
    nc = _get_program()
    res = run_bass_kernel_spmd(nc, in_maps, list(range(N_CORES))).results
    out_a = np.empty((B, S, H), np.float32)
    out_b = np.empty((B, S, H), np.float32)
    for b_i in range(B):
        out_a[b_i] = res[2 * b_i]["out_s0"] + res[2 * b_i + 1]["out_s0"] + vshifts[0]
        out_b[b_i] = res[2 * b_i]["out_s1"] + res[2 * b_i + 1]["out_s1"] + vshifts[1]
    return out_a, out_b


def _get_program():
    global _PROGRAM
    if _PROGRAM is None:
        _PROGRAM = _build_program()
    return _PROGRAM


# revision 22
# speedup vs baseline: 1.0946x; 1.0584x over previous
"""DualStreamEncoderAttention Trainium2 kernel (v3).

Sharding: 8 cores = 4 samples x 2 head-groups (8 heads each). Each core
computes, for its sample, both streams' QKV(+RoPE) for its 8 heads,
cross-stream attention, and a partial out-projection over its heads'
rows of Wout. The host sums the two partial projections per sample.

v3 speed strategy (S=1024, H=1024, D=64, 8 local heads):
  - PV reformulated with es as the matmul stationary ([128k x 128q]
    slices) and [v | ones] as the 65-wide moving operand: the PE pays 65
    columns per 128x128x65 block instead of 512 columns for 65 output
    rows. attn lands in [q, hd] orientation; per-query softmax
    denominators live on the partition axis, so normalization is a plain
    per-partition tensor_scalar multiply.
  - attn [q, hd] -> [hd, q] via 128x128 XBAR dma transposes (bf16), out
    projection is a bf16 single-accumulation pass per [128S x 512oc]
    tile, split into an early (heads 0-3) and late (heads 4-7) partial
    merged on DVE.
  - softmax exp is split across engines: 12 of 16 key-chunks per head
    run fp8-DoubleRow QK^T + exact exp on the Activation engine; 4 run
    bf16 QK^T (from bf16 rope copies) + a Schraudolph int16/bf16
    bit-pun exp on the Vector engine. The pun's bounded sawtooth error
    (+-4%) replaces the fp8 score error on those chunks, keeping overall
    relmax at baseline while cutting ACT exp work by 25%.
  - LayerNorm folded on host (xhat pre-transposed, gamma into W, beta
    via a per-block scalar add + host-side V-shift correction).
"""

import sys

for _p in ("/opt/trn_rl_repo", "/root/.axon_site/_ro/trn_rl_repo"):
    if _p not in sys.path:
        sys.path.insert(0, _p)

import numpy as np

S = 1024
H = 1024
NH = 16
D = 64
NHL = 8          # heads per core
P = 128
N_CORES = 8
LN_EPS = 1e-5
ROPE_BASE = 10000.0
SCALE = float(D) ** -0.5

# fast-exp (Schraudolph bf16 pun) constants: i16 = rint(A*score + B),
# bitcast bf16 ~= exp(score*SCALE) * 2^-sigma residual, sigma minimax.
LOG2E = 1.4426950408889634
FX_SIGMA = 0.0579
A_FX = SCALE * LOG2E * 128.0
B_FX = 16256.0 - 128.0 * FX_SIGMA
FX_TST = (0, 1)   # key-chunks tst<2 of each target stream use the pun path

_PROGRAM = None


def _rope_tables(height, width, head_dim=D):
    """Mirror of reference.rope_2d_tables in numpy float32."""
    height = int(height)
    width = int(width)
    dim_x = head_dim // 2
    dim_y = head_dim - dim_x
    inv_fx = 1.0 / (ROPE_BASE ** (np.arange(0, dim_x, 2, dtype=np.float32) / np.float32(dim_x)))
    inv_fy = 1.0 / (ROPE_BASE ** (np.arange(0, dim_y, 2, dtype=np.float32) / np.float32(dim_y)))
    fx = np.arange(width, dtype=np.float32)[:, None] * inv_fx[None, :]
    fy = np.arange(height, dtype=np.float32)[:, None] * inv_fy[None, :]
    fx = np.concatenate([fx, fx], axis=-1)  # [W, dim_x]
    fy = np.concatenate([fy, fy], axis=-1)  # [H, dim_y]
    cos = np.concatenate([
        np.broadcast_to(np.cos(fx)[None, :, :], (height, width, dim_x)),
        np.broadcast_to(np.cos(fy)[:, None, :], (height, width, dim_y)),
    ], axis=-1).reshape(height * width, head_dim).astype(np.float32)
    sin = np.concatenate([
        np.broadcast_to(np.sin(fx)[None, :, :], (height, width, dim_x)),
        np.broadcast_to(np.sin(fy)[:, None, :], (height, width, dim_y)),
    ], axis=-1).reshape(height * width, head_dim).astype(np.float32)
    return cos, sin


def _build_program(do_compile=True):
    import concourse.mybir as mybir
    import concourse.tile as tile
    from concourse import bacc

    f32 = mybir.dt.float32
    bf16 = mybir.dt.bfloat16
    fp8 = mybir.dt.float8e4
    i16 = mybir.dt.int16
    DR = mybir.MatmulPerfMode.DoubleRow
    AF = mybir.ActivationFunctionType
    ALU = mybir.AluOpType

    nc = bacc.Bacc("TRN2")

    # ---- DRAM parameters (per-core tensors; same program on all cores) ----
    xh_d = [nc.dram_tensor(f"xh_s{s}", [H, S], bf16, kind="ExternalInput") for s in range(2)]
    wqk_d = [nc.dram_tensor(f"wqk_s{s}", [H, 2 * NHL * D], bf16, kind="ExternalInput") for s in range(2)]
    wv_d = [nc.dram_tensor(f"wv_s{s}", [H, NHL * D], bf16, kind="ExternalInput") for s in range(2)]
    cqk_d = [nc.dram_tensor(f"cqk_s{s}", [P, 8], f32, kind="ExternalInput") for s in range(2)]
    wout_d = [nc.dram_tensor(f"wout_s{s}", [NHL * D, H], bf16, kind="ExternalInput") for s in range(2)]
    cos_d = nc.dram_tensor("cosdr", [P, 2 * S], bf16, kind="ExternalInput")
    sin_d = nc.dram_tensor("sindr", [P, 2 * S], bf16, kind="ExternalInput")  # pre-negated i=0 half
    out_d = [nc.dram_tensor(f"out_s{s}", [S, H], f32, kind="ExternalOutput") for s in range(2)]

    with tile.TileContext(nc) as tc:
        with (
            tc.tile_pool(name="consts", bufs=1) as consts,
            tc.tile_pool(name="persist", bufs=1) as persist,
            tc.tile_pool(name="esb", bufs=4) as esb_p,
            tc.tile_pool(name="small", bufs=2) as small,
            tc.tile_pool(name="mmp", bufs=2, space="PSUM") as mmp,
            tc.tile_pool(name="scp", bufs=2, space="PSUM") as scp,
            tc.tile_pool(name="accp", bufs=1, space="PSUM") as accp,
        ):
            cosdr = consts.tile([P, 2, S], bf16, tag="cosdr")
            sindr = consts.tile([P, 2, S], bf16, tag="sindr")
            with tc.high_priority(offset=-60):
                nc.sync.dma_start(out=cosdr, in_=cos_d[:].rearrange("p (i s) -> p i s", i=2))
                nc.sync.dma_start(out=sindr, in_=sin_d[:].rearrange("p (i s) -> p i s", i=2))
            zeroc = consts.tile([P, 1], f32, tag="zeroc")
            nc.vector.memset(zeroc, 0.0)
            # warm the PE p-state while the first DMAs land
            warm = consts.tile([P, 512], f32, tag="warm")
            nc.vector.memset(warm, 0.0)
            wps = mmp.tile([P, 512], f32, tag="mm", name="wps")
            for wi in range(4):
                nc.tensor.matmul(wps[0:1, :], zeroc, warm, start=(wi == 0), stop=(wi == 3))

            # persistent per-stream state
            qdr = [[persist.tile([P, 2, S], fp8, tag=f"qdr{s}_{ht}", name=f"qdr{s}_{ht}")
                    for ht in range(2)] for s in range(2)]
            kdr = [[persist.tile([P, 2, S], fp8, tag=f"kdr{s}_{ht}", name=f"kdr{s}_{ht}")
                    for ht in range(2)] for s in range(2)]
            # bf16 rope output, head-pair packed: pair tile [128 = (a,i)*32+dl, S]
            qbf = [[[persist.tile([P, S], bf16, tag=f"qbf{s}_{ht}_{pr}", name=f"qbf{s}_{ht}_{pr}")
                     for pr in range(2)] for ht in range(2)] for s in range(2)]
            kbf = [[[persist.tile([P, len(FX_TST) * P], bf16, tag=f"kbf{s}_{ht}_{pr}", name=f"kbf{s}_{ht}_{pr}")
                     for pr in range(2)] for ht in range(2)] for s in range(2)]
            v_sb = [[persist.tile([P, NHL, D + 1], bf16, tag=f"v{s}_{st}", name=f"v{s}_{st}")
                     for st in range(8)] for s in range(2)]
            attn2 = [[persist.tile([P, NHL * D, ], bf16, tag=f"at2_{s}_{qc}", name=f"at2_{s}_{qc}")
                      for qc in range(8)] for s in range(2)]
            attnT = [[persist.tile([P, S], bf16, tag=f"atT_{s}_{p}", name=f"atT_{s}_{p}")
                      for p in range(4)] for s in range(2)]

            # ---------------- prep helpers ----------------
            def load_stream(s, prep_p):
                xh = [prep_p.tile([P, S], bf16, tag=f"xh{s}_{hc}", name=f"xh{s}_{hc}") for hc in range(8)]
                qs = [nc.sync, nc.gpsimd, nc.scalar]
                for hc in range(8):
                    qs[hc % 3].dma_start(out=xh[hc], in_=xh_d[s][hc * P:(hc + 1) * P, :])
                cqk_sb = prep_p.tile([P, 8], f32, tag="cqk", bufs=2, name="cqk_sb")
                nc.sync.dma_start(out=cqk_sb, in_=cqk_d[s][:])
                return xh, cqk_sb

            def qk_group(s, xh, cqk_sb, ht, prep_p):
                """q+k projections, rope, fp8 DR tiles + bf16 side copies."""
                wqfs = {}
                for qk in range(2):
                    for half in range(2):
                        b = qk * 4 + ht * 2 + half
                        wqf = prep_p.tile([P, 8, P], bf16, tag="wqf", bufs=4, name="wqf")
                        (nc.sync if (qk + half) % 2 == 0 else nc.gpsimd).dma_start(
                            out=wqf,
                            in_=wqk_d[s][:, b * P:(b + 1) * P].rearrange("(c p) n -> p c n", p=P))
                        wqfs[(qk, half)] = wqf
                # bf16 rope staging (DR partition layout)
                qbfdr = prep_p.tile([P, 2, S], bf16, tag="qbfdr", bufs=2, name="qbfdr")
                kbfdr = prep_p.tile([P, 2, len(FX_TST) * P], bf16, tag="kbfdr", bufs=2, name="kbfdr")
                nfx = len(FX_TST) * P
                for sc in range(2):
                    csl = slice(sc * 512, (sc + 1) * 512)
                    for qk in range(2):
                        dst = (qdr if qk == 0 else kdr)[s]
                        stg_t = [None, None]
                        for half in range(2):
                            b = qk * 4 + ht * 2 + half
                            psq = mmp.tile([P, 512], f32, tag="mm", name="psq")
                            for kc in range(8):
                                nc.tensor.matmul(
                                    psq,
                                    wqfs[(qk, half)][:, kc, :],
                                    xh[kc][:, csl],
                                    start=(kc == 0), stop=(kc == 7),
                                )
                            stg = prep_p.tile([P, 512], bf16, tag="stg", bufs=5, name="stg")
                            nc.vector.tensor_scalar_add(stg, psq, cqk_sb[:, b:b + 1])
                            stg_t[half] = stg
                        for i in range(2):
                            tmp = small.tile([P, 512], bf16, tag="rtmp", bufs=3, name="rtmp")
                            nc.gpsimd.tensor_mul(tmp, stg_t[1 - i], sindr[:, i, csl])
                            qc = small.tile([P, 512], bf16, tag="rqc", bufs=3, name="rqc")
                            nc.gpsimd.tensor_mul(qc, stg_t[i], cosdr[:, i, csl])
                            nc.vector.tensor_add(dst[ht][:, i, csl], tmp, qc)
                            # bf16 side copies for the fast-exp score path
                            if qk == 0:
                                nc.gpsimd.tensor_add(qbfdr[:, i, csl], tmp, qc)
                            elif sc == 0:
                                nc.gpsimd.tensor_add(kbfdr[:, i, 0:nfx],
                                                     tmp[:, 0:nfx], qc[:, 0:nfx])
                # partition shuffle DR layout -> head-pair [64|64] layout
                for pr in range(2):
                    for a in range(2):
                        hh = 2 * pr + a
                        for i in range(2):
                            po = slice(64 * a + 32 * i, 64 * a + 32 * i + 32)
                            pi = slice(32 * hh, 32 * hh + 32)
                            nc.gpsimd.dma_start(out=qbf[s][ht][pr][po, :], in_=qbfdr[pi, i, :])
                            nc.sync.dma_start(out=kbf[s][ht][pr][po, :], in_=kbfdr[pi, i, :])

            def v_load(s, prep_p):
                wvf = prep_p.tile([P, 8, NHL * D], bf16, tag=f"wvf{s}", name="wvf")
                nc.gpsimd.dma_start(out=wvf, in_=wv_d[s][:].rearrange("(c p) n -> p c n", p=P))
                for st in range(8):
                    nc.gpsimd.memset(v_sb[s][st][:, :, D:D + 1], 1.0)
                return wvf

            def v_fills(s, xh, wvf, halves=(0, 1)):
                for nh in halves:
                    nsl = slice(nh * 256, (nh + 1) * 256)
                    for st in range(8):
                        psv = mmp.tile([P, 512], f32, tag="mm", name="psv")
                        for kc in range(8):
                            nc.tensor.matmul(
                                psv[:, 0:256],
                                xh[kc][:, st * P:(st + 1) * P],
                                wvf[:, kc, nsl],
                                start=(kc == 0), stop=(kc == 7),
                            )
                        nc.vector.tensor_copy(
                            out=v_sb[s][st][:, 4 * nh:4 * nh + 4, 0:D],
                            in_=psv[:, 0:256].rearrange("p (h d) -> p h d", d=D),
                        )

            # ---------------- attention per head (all 16 key-chunks) ---------
            # spread the 4 fast-exp chunks through the head; lead with an
            # ACT chunk so the PV accumulation group never waits on DVE
            TST_ORDER = (2, 0, 3, 4, 5, 1, 6, 7)

            acc_live = {}

            def head_attn(s, ht, hh, ts_range=(0, 1)):
                h = 4 * ht + hh
                pr = slice(32 * hh, 32 * hh + 32)
                pair, a = hh // 2, hh % 2
                pp64 = slice(64 * a, 64 * a + 64)
                if 0 in ts_range:
                    accs = [accp.tile([P, 4, D + 1], f32, tag="accA", name="accA"),
                            accp.tile([P, 4, D + 1], f32, tag="accB", name="accB")]
                    acc_live[(s, h)] = accs
                else:
                    accs = acc_live[(s, h)]
                for cidx, (ts, tst) in enumerate(
                        (t, o) for t in range(2) for o in TST_ORDER):
                    if ts not in ts_range:
                        continue
                    first, last = cidx == 0, cidx == 15
                    sc_ps = scp.tile([P, S], f32, tag="sc", name="sc_ps")
                    if tst in FX_TST:
                        # bf16 scores + vector-engine fast-exp pun
                        for sc in range(2):
                            csl = slice(sc * 512, (sc + 1) * 512)
                            nc.tensor.matmul(
                                sc_ps[:, csl],
                                kbf[ts][ht][pair][pp64, tst * P:(tst + 1) * P],
                                qbf[s][ht][pair][pp64, csl],
                            )
                        esx = esb_p.tile([P, S], i16, tag="esx", bufs=4, name="esx")
                        with tc.high_priority(offset=-40):
                            nc.vector.tensor_scalar(
                                out=esx, in0=sc_ps,
                                scalar1=A_FX, scalar2=B_FX,
                                op0=ALU.mult, op1=ALU.add,
                            )
                        es = esx.bitcast(bf16)
                    else:
                        for sc in range(2):
                            csl = slice(sc * 512, (sc + 1) * 512)
                            nc.tensor.matmul(
                                sc_ps[:, csl],
                                kdr[ts][ht][pr, :, tst * P:(tst + 1) * P],
                                qdr[s][ht][pr, :, csl],
                                perf_mode=DR,
                                tile_position=(32 * hh, 0),
                            )
                        est = esb_p.tile([P, S], bf16, tag="es", bufs=9, name="es")
                        nc.scalar.activation(out=est, in_=sc_ps, func=AF.Exp,
                                             bias=zeroc, scale=SCALE)
                        es = est
                    with tc.high_priority(offset=-25):
                        for qch in range(8):
                            nc.tensor.matmul(
                                accs[qch // 4][:, qch % 4, :],
                                es[:, qch * P:(qch + 1) * P],
                                v_sb[ts][tst][:, h, :],
                                start=first, stop=last,
                            )
                if 1 not in ts_range:
                    return
                # normalize: per-q denominators sit on the free axis (col 64)
                for grp in range(2):
                    rr = small.tile([P, 4], f32, tag="rr", bufs=2, name="rr")
                    nc.vector.reciprocal(out=rr, in_=accs[grp][:, :, D])
                    for j in range(4):
                        qch = grp * 4 + j
                        nc.vector.tensor_scalar_mul(
                            attn2[s][qch][:, h * D:(h + 1) * D],
                            accs[grp][:, j, 0:D],
                            rr[:, j:j + 1],
                        )

            def transposes(s, ps):
                for p in ps:
                    for qch in range(8):
                        nc.sync.dma_start_transpose(
                            out=attnT[s][p][:, qch * P:(qch + 1) * P],
                            in_=attn2[s][qch][:, p * P:(p + 1) * P],
                        )

            # ---------------- out-projection ----------------
            opar = {}

            def tail_a(s, wop, wo_t):
                for p in range(2):
                    nc.sync.dma_start(out=wo_t[p], in_=wout_d[s][p * P:(p + 1) * P, :])
                for st in range(8):
                    for oc in range(2):
                        pso = mmp.tile([P, 512], f32, tag="mm", name="pso")
                        for p in range(2):
                            nc.tensor.matmul(
                                pso,
                                attnT[s][p][:, st * P:(st + 1) * P],
                                wo_t[p][:, oc * 512:(oc + 1) * 512],
                                start=(p == 0), stop=(p == 1),
                            )
                        op_t = wop.tile([P, 512], bf16, tag=f"opar{st}_{oc}", name="opar")
                        nc.vector.tensor_copy(out=op_t, in_=pso)
                        opar[(s, st, oc)] = op_t

            def tail_b(s, wop, wo_t):
                for p in range(2, 4):
                    nc.sync.dma_start(out=wo_t[p], in_=wout_d[s][p * P:(p + 1) * P, :])
                for st in range(8):
                    for oc in range(2):
                        pso = mmp.tile([P, 512], f32, tag="mm", name="pso")
                        for p in range(2, 4):
                            nc.tensor.matmul(
                                pso,
                                attnT[s][p][:, st * P:(st + 1) * P],
                                wo_t[p][:, oc * 512:(oc + 1) * 512],
                                start=(p == 2), stop=(p == 3),
                            )
                        osb = small.tile([P, 512], f32, tag="osb", bufs=3, name="osb")
                        nc.vector.tensor_add(osb, pso, opar[(s, st, oc)])
                        (nc.gpsimd if (st + oc) % 2 == 0 else nc.sync).dma_start(
                            out=out_d[s][st * P:(st + 1) * P, oc * 512:(oc + 1) * 512], in_=osb)

            # ---------------- emission ----------------
            with tc.tile_pool(name="prep", bufs=1) as prep_p:
                xh0, cq0 = load_stream(0, prep_p)
                qk_group(0, xh0, cq0, 0, prep_p)
                wvf0 = v_load(0, prep_p)
                with tc.high_priority(offset=-50):
                    v_fills(0, xh0, wvf0, halves=(0,))
                xh1, cq1 = load_stream(1, prep_p)
                head_attn(0, 0, 0, ts_range=(0,))
                qk_group(1, xh1, cq1, 0, prep_p)
                wvf1 = v_load(1, prep_p)
                v_fills(1, xh1, wvf1, halves=(0,))
                head_attn(0, 0, 0, ts_range=(1,))
                for hh in range(1, 4):
                    head_attn(0, 0, hh)
                qk_group(0, xh0, cq0, 1, prep_p)
                v_fills(0, xh0, wvf0, halves=(1,))
                for hh in range(4):
                    head_attn(1, 0, hh)
                qk_group(1, xh1, cq1, 1, prep_p)
                v_fills(1, xh1, wvf1, halves=(1,))
                for hh in range(4):
                    head_attn(0, 1, hh)
            with tc.tile_pool(name="wo", bufs=1) as wop:
                wo_t0 = [wop.tile([P, H], bf16, tag=f"wo{p}", name=f"wo{p}") for p in range(4)]
                wo_t1 = [wop.tile([P, H], bf16, tag=f"wo{p}", name=f"wo{p}") for p in range(4)]
                transposes(0, (0, 1, 2, 3))
                tail_a(0, wop, wo_t0)
                tail_b(0, wop, wo_t0)
                transposes(1, (0, 1))
                tail_a(1, wop, wo_t1)
                for hh in range(4):
                    head_attn(1, 1, hh)
                transposes(1, (2, 3))
                tail_b(1, wop, wo_t1)

    if do_compile:
        nc.compile()
    return nc


def _host_prep(x_a, x_b, Wqkv_a, Wqkv_b, Wout_a, Wout_b,
               gamma_a, beta_a, gamma_b, beta_b, height, width):
    """Build the 8 per-core input maps."""
    import ml_dtypes
    cos, sin = _rope_tables(height, width)      # [S, 64]

    # DR rope tables [128, 2, S]: partition p = 32*hh + dl (repeats over hh)
    dl = np.arange(32)
    cos_dr = np.empty((P, 2, S), np.float32)
    sin_dr = np.empty((P, 2, S), np.float32)
    for hh in range(4):
        rows = 32 * hh + dl
        cos_dr[rows, 0, :] = cos[:, dl].T          # cos[s, dl]
        cos_dr[rows, 1, :] = cos[:, 32 + dl].T
        sin_dr[rows, 0, :] = -sin[:, dl].T         # pre-negated for i=0
        sin_dr[rows, 1, :] = sin[:, 32 + dl].T
    cos_dr = np.ascontiguousarray(cos_dr.reshape(P, 2 * S).astype(ml_dtypes.bfloat16))
    sin_dr = np.ascontiguousarray(sin_dr.reshape(P, 2 * S).astype(ml_dtypes.bfloat16))

    # host LayerNorm fold: xhat = r*(x-mu), shipped transposed [H, S]
    def xhat(x):
        x = x.astype(np.float32)
        mu = x.mean(axis=-1, keepdims=True)
        var = ((x - mu) ** 2).mean(axis=-1, keepdims=True)
        r = 1.0 / np.sqrt(var + LN_EPS)
        return ((x - mu) * r).astype(np.float32)

    streams = []
    vshifts = []
    for (W, Wo, g, b) in ((Wqkv_a, Wout_a, gamma_a, beta_a), (Wqkv_b, Wout_b, gamma_b, beta_b)):
        Wg = (W * g[:, None]).astype(np.float32)       # gamma-folded
        cfull = (b.astype(np.float64) @ W.astype(np.float64)).astype(np.float32)  # beta@W [3H]
        W4 = Wg.reshape(H, 3, NH, D)
        c4 = cfull.reshape(3, NH, D)
        per_hg = []
        for hg in range(2):
            h0 = hg * NHL
            # DR column order: blocks (qk, ht, half) of 128 cols = (hh, dl)
            cols = []
            ccols = []
            for qk in range(2):
                for ht in range(2):
                    for half in range(2):
                        for hh in range(4):
                            head = h0 + 4 * ht + hh
                            dsl = slice(32 * half, 32 * half + 32)
                            cols.append(W4[:, qk, head, dsl])      # [H, 32]
                            ccols.append(c4[qk, head, dsl])        # [32]
            wqk = np.ascontiguousarray(np.concatenate(cols, axis=1).astype(ml_dtypes.bfloat16))
            # beta@W per qk column as per-partition scalars [128, 8 blocks]
            cqk_blk = np.ascontiguousarray(
                np.concatenate(ccols).reshape(8, P).T.astype(np.float32))   # [128, 8]
            wv = np.ascontiguousarray(W4[:, 2, h0:h0 + NHL, :].reshape(H, NHL * D).astype(ml_dtypes.bfloat16))
            wout = np.ascontiguousarray(
                Wo.reshape(NH, D, H)[h0:h0 + NHL].reshape(NHL * D, H).astype(ml_dtypes.bfloat16))
            per_hg.append(dict(wqk=wqk, wv=wv, cqk=cqk_blk, wout=wout))
        # exact host-side V correction: beta@Wv shifts attn uniformly
        # (softmax weights sum to 1), so it lands as a constant row on out
        vshift = (cfull[2 * H:3 * H].astype(np.float64) @ Wo.astype(np.float64)).astype(np.float32)
        streams.append(per_hg)
        vshifts.append(vshift)

    in_maps = []
    B = x_a.shape[0]
    xh_a = [np.ascontiguousarray(xhat(x_a[b_i]).T.astype(ml_dtypes.bfloat16)) for b_i in range(B)]
    xh_b = [np.ascontiguousarray(xhat(x_b[b_i]).T.astype(ml_dtypes.bfloat16)) for b_i in range(B)]
    for c in range(N_CORES):
        b_i, hg = (c // 2) % B, c % 2
        m = {
            "xh_s0": xh_a[b_i],
            "xh_s1": xh_b[b_i],
            "cosdr": cos_dr, "sindr": sin_dr,
        }
        for s in range(2):
            blk = streams[s][hg]
            m[f"wqk_s{s}"] = blk["wqk"]
            m[f"wv_s{s}"] = blk["wv"]
            m[f"cqk_s{s}"] = blk["cqk"]
            m[f"wout_s{s}"] = blk["wout"]
        in_maps.append(m)
    return in_maps, vshifts


def kernel(x_a, x_b, Wqkv_a, Wqkv_b, Wout_a, Wout_b,
           gamma_a, beta_a, gamma_b, beta_b, height, width):
    from concourse.bass_utils import run_bass_kernel_spmd

    x_a = np.asarray(x_a, dtype=np.float32)
    x_b = np.asarray(x_b, dtype=np.float32)
    B = x_a.shape[0]
    in_maps, vshifts = _host_prep(x_a, x_b,
                         np.asarray(Wqkv_a, np.float32), np.asarray(Wqkv_b, np.float32),
                         np.asarray(Wout_a, np.float32), np.asarray(Wout_b, np.float32),
                         np.asarray(gamma_a, np.float32), np.asarray(beta_a, np.float32),
                         np.asarray(gamma_b, np.float32), np.asarray(beta_b, np.float32),
                         height, width)
    nc = _get_program()
    res = run_bass_kernel_spmd(nc, in_maps, list(range(N_CORES))).results
    out_a = np.empty((B, S, H), np.float32)
    out_b = np.empty((B, S, H), np.float32)
    for b_i in range(B):
        out_a[b_i] = res[2 * b_i]["out_s0"] + res[2 * b_i + 1]["out_s0"] + vshifts[0]
        out_b[b_i] = res[2 * b_i]["out_s1"] + res[2 * b_i + 1]["out_s1"] + vshifts[1]
    return out_a, out_b


def _get_program():
    global _PROGRAM
    if _PROGRAM is None:
        _PROGRAM = _build_program()
    return _PROGRAM


# revision 24
# speedup vs baseline: 1.0964x; 1.0016x over previous
"""DualStreamEncoderAttention Trainium2 kernel (v3).

Sharding: 8 cores = 4 samples x 2 head-groups (8 heads each). Each core
computes, for its sample, both streams' QKV(+RoPE) for its 8 heads,
cross-stream attention, and a partial out-projection over its heads'
rows of Wout. The host sums the two partial projections per sample.

v3 speed strategy (S=1024, H=1024, D=64, 8 local heads):
  - PV reformulated with es as the matmul stationary ([128k x 128q]
    slices) and [v | ones] as the 65-wide moving operand: the PE pays 65
    columns per 128x128x65 block instead of 512 columns for 65 output
    rows. attn lands in [q, hd] orientation; per-query softmax
    denominators live on the partition axis, so normalization is a plain
    per-partition tensor_scalar multiply.
  - attn [q, hd] -> [hd, q] via 128x128 XBAR dma transposes (bf16), out
    projection is a bf16 single-accumulation pass per [128S x 512oc]
    tile, split into an early (heads 0-3) and late (heads 4-7) partial
    merged on DVE.
  - softmax exp is split across engines: 12 of 16 key-chunks per head
    run fp8-DoubleRow QK^T + exact exp on the Activation engine; 4 run
    bf16 QK^T (from bf16 rope copies) + a Schraudolph int16/bf16
    bit-pun exp on the Vector engine. The pun's bounded sawtooth error
    (+-4%) replaces the fp8 score error on those chunks, keeping overall
    relmax at baseline while cutting ACT exp work by 25%.
  - LayerNorm folded on host (xhat pre-transposed, gamma into W, beta
    via a per-block scalar add + host-side V-shift correction).
"""

import sys

for _p in ("/opt/trn_rl_repo", "/root/.axon_site/_ro/trn_rl_repo"):
    if _p not in sys.path:
        sys.path.insert(0, _p)

import numpy as np

S = 1024
H = 1024
NH = 16
D = 64
NHL = 8          # heads per core
P = 128
N_CORES = 8
LN_EPS = 1e-5
ROPE_BASE = 10000.0
SCALE = float(D) ** -0.5

# fast-exp (Schraudolph bf16 pun) constants: i16 = rint(A*score + B),
# bitcast bf16 ~= exp(score*SCALE) * 2^-sigma residual, sigma minimax.
LOG2E = 1.4426950408889634
FX_SIGMA = 0.0579
A_FX = SCALE * LOG2E * 128.0
B_FX = 16256.0 - 128.0 * FX_SIGMA
FX_TST = (0, 1)   # key-chunks tst<2 of each target stream use the pun path

_PROGRAM = None


def _rope_tables(height, width, head_dim=D):
    """Mirror of reference.rope_2d_tables in numpy float32."""
    height = int(height)
    width = int(width)
    dim_x = head_dim // 2
    dim_y = head_dim - dim_x
    inv_fx = 1.0 / (ROPE_BASE ** (np.arange(0, dim_x, 2, dtype=np.float32) / np.float32(dim_x)))
    inv_fy = 1.0 / (ROPE_BASE ** (np.arange(0, dim_y, 2, dtype=np.float32) / np.float32(dim_y)))
    fx = np.arange(width, dtype=np.float32)[:, None] * inv_fx[None, :]
    fy = np.arange(height, dtype=np.float32)[:, None] * inv_fy[None, :]
    fx = np.concatenate([fx, fx], axis=-1)  # [W, dim_x]
    fy = np.concatenate([fy, fy], axis=-1)  # [H, dim_y]
    cos = np.concatenate([
        np.broadcast_to(np.cos(fx)[None, :, :], (height, width, dim_x)),
        np.broadcast_to(np.cos(fy)[:, None, :], (height, width, dim_y)),
    ], axis=-1).reshape(height * width, head_dim).astype(np.float32)
    sin = np.concatenate([
        np.broadcast_to(np.sin(fx)[None, :, :], (height, width, dim_x)),
        np.broadcast_to(np.sin(fy)[:, None, :], (height, width, dim_y)),
    ], axis=-1).reshape(height * width, head_dim).astype(np.float32)
    return cos, sin


def _build_program(do_compile=True):
    import concourse.mybir as mybir
    import concourse.tile as tile
    from concourse import bacc

    f32 = mybir.dt.float32
    bf16 = mybir.dt.bfloat16
    fp8 = mybir.dt.float8e4
    i16 = mybir.dt.int16
    DR = mybir.MatmulPerfMode.DoubleRow
    AF = mybir.ActivationFunctionType
    ALU = mybir.AluOpType

    nc = bacc.Bacc("TRN2")

    # ---- DRAM parameters (per-core tensors; same program on all cores) ----
    xh_d = [nc.dram_tensor(f"xh_s{s}", [H, S], bf16, kind="ExternalInput") for s in range(2)]
    wqk_d = [nc.dram_tensor(f"wqk_s{s}", [H, 2 * NHL * D], bf16, kind="ExternalInput") for s in range(2)]
    wv_d = [nc.dram_tensor(f"wv_s{s}", [H, NHL * D], bf16, kind="ExternalInput") for s in range(2)]
    cqk_d = [nc.dram_tensor(f"cqk_s{s}", [P, 8], f32, kind="ExternalInput") for s in range(2)]
    wout_d = [nc.dram_tensor(f"wout_s{s}", [NHL * D, H], bf16, kind="ExternalInput") for s in range(2)]
    cos_d = nc.dram_tensor("cosdr", [P, 2 * S], bf16, kind="ExternalInput")
    sin_d = nc.dram_tensor("sindr", [P, 2 * S], bf16, kind="ExternalInput")  # pre-negated i=0 half
    out_d = [nc.dram_tensor(f"out_s{s}", [S, H], f32, kind="ExternalOutput") for s in range(2)]

    with tile.TileContext(nc) as tc:
        with (
            tc.tile_pool(name="consts", bufs=1) as consts,
            tc.tile_pool(name="persist", bufs=1) as persist,
            tc.tile_pool(name="esb", bufs=4) as esb_p,
            tc.tile_pool(name="small", bufs=2) as small,
            tc.tile_pool(name="mmp", bufs=2, space="PSUM") as mmp,
            tc.tile_pool(name="scp", bufs=2, space="PSUM") as scp,
            tc.tile_pool(name="accp", bufs=1, space="PSUM") as accp,
        ):
            cosdr = consts.tile([P, 2, S], bf16, tag="cosdr")
            sindr = consts.tile([P, 2, S], bf16, tag="sindr")
            with tc.high_priority(offset=-60):
                nc.sync.dma_start(out=cosdr, in_=cos_d[:].rearrange("p (i s) -> p i s", i=2))
                nc.sync.dma_start(out=sindr, in_=sin_d[:].rearrange("p (i s) -> p i s", i=2))
            zeroc = consts.tile([P, 1], f32, tag="zeroc")
            nc.vector.memset(zeroc, 0.0)
            # warm the PE p-state while the first DMAs land
            warm = consts.tile([P, 512], f32, tag="warm")
            nc.vector.memset(warm, 0.0)
            wps = mmp.tile([P, 512], f32, tag="mm", name="wps")
            for wi in range(4):
                nc.tensor.matmul(wps[0:1, :], zeroc, warm, start=(wi == 0), stop=(wi == 3))

            # persistent per-stream state
            qdr = [[persist.tile([P, 2, S], fp8, tag=f"qdr{s}_{ht}", name=f"qdr{s}_{ht}")
                    for ht in range(2)] for s in range(2)]
            kdr = [[persist.tile([P, 2, S], fp8, tag=f"kdr{s}_{ht}", name=f"kdr{s}_{ht}")
                    for ht in range(2)] for s in range(2)]
            # bf16 rope output, head-pair packed: pair tile [128 = (a,i)*32+dl, S]
            qbf = [[[persist.tile([P, S], bf16, tag=f"qbf{s}_{ht}_{pr}", name=f"qbf{s}_{ht}_{pr}")
                     for pr in range(2)] for ht in range(2)] for s in range(2)]
            kbf = [[[persist.tile([P, len(FX_TST) * P], bf16, tag=f"kbf{s}_{ht}_{pr}", name=f"kbf{s}_{ht}_{pr}")
                     for pr in range(2)] for ht in range(2)] for s in range(2)]
            v_sb = [[persist.tile([P, NHL, D + 1], bf16, tag=f"v{s}_{st}", name=f"v{s}_{st}")
                     for st in range(8)] for s in range(2)]
            attn2 = [[persist.tile([P, NHL * D, ], bf16, tag=f"at2_{s}_{qc}", name=f"at2_{s}_{qc}")
                      for qc in range(8)] for s in range(2)]
            attnT = [[persist.tile([P, S], bf16, tag=f"atT_{s}_{p}", name=f"atT_{s}_{p}")
                      for p in range(4)] for s in range(2)]

            # ---------------- prep helpers ----------------
            def load_stream(s, prep_p):
                xh = [prep_p.tile([P, S], bf16, tag=f"xh{s}_{hc}", name=f"xh{s}_{hc}") for hc in range(8)]
                qs = [nc.sync, nc.gpsimd, nc.scalar]
                for hc in range(8):
                    qs[hc % 3].dma_start(out=xh[hc], in_=xh_d[s][hc * P:(hc + 1) * P, :])
                cqk_sb = prep_p.tile([P, 8], f32, tag="cqk", bufs=2, name="cqk_sb")
                nc.sync.dma_start(out=cqk_sb, in_=cqk_d[s][:])
                return xh, cqk_sb

            def qk_group(s, xh, cqk_sb, ht, prep_p):
                """q+k projections, rope, fp8 DR tiles + bf16 side copies."""
                wqfs = {}
                for qk in range(2):
                    for half in range(2):
                        b = qk * 4 + ht * 2 + half
                        wqf = prep_p.tile([P, 8, P], bf16, tag="wqf", bufs=4, name="wqf")
                        (nc.sync if (qk + half) % 2 == 0 else nc.gpsimd).dma_start(
                            out=wqf,
                            in_=wqk_d[s][:, b * P:(b + 1) * P].rearrange("(c p) n -> p c n", p=P))
                        wqfs[(qk, half)] = wqf
                # bf16 rope staging (DR partition layout)
                qbfdr = prep_p.tile([P, 2, S], bf16, tag="qbfdr", bufs=2, name="qbfdr")
                kbfdr = prep_p.tile([P, 2, len(FX_TST) * P], bf16, tag="kbfdr", bufs=2, name="kbfdr")
                nfx = len(FX_TST) * P
                for sc in range(2):
                    csl = slice(sc * 512, (sc + 1) * 512)
                    for qk in range(2):
                        dst = (qdr if qk == 0 else kdr)[s]
                        stg_t = [None, None]
                        for half in range(2):
                            b = qk * 4 + ht * 2 + half
                            psq = mmp.tile([P, 512], f32, tag="mm", name="psq")
                            for kc in range(8):
                                nc.tensor.matmul(
                                    psq,
                                    wqfs[(qk, half)][:, kc, :],
                                    xh[kc][:, csl],
                                    start=(kc == 0), stop=(kc == 7),
                                )
                            stg = prep_p.tile([P, 512], bf16, tag="stg", bufs=5, name="stg")
                            nc.vector.tensor_scalar_add(stg, psq, cqk_sb[:, b:b + 1])
                            stg_t[half] = stg
                        for i in range(2):
                            tmp = small.tile([P, 512], bf16, tag="rtmp", bufs=3, name="rtmp")
                            nc.gpsimd.tensor_mul(tmp, stg_t[1 - i], sindr[:, i, csl])
                            qc = small.tile([P, 512], bf16, tag="rqc", bufs=3, name="rqc")
                            nc.gpsimd.tensor_mul(qc, stg_t[i], cosdr[:, i, csl])
                            nc.vector.tensor_add(dst[ht][:, i, csl], tmp, qc)
                            # bf16 side copies for the fast-exp score path
                            if qk == 0:
                                nc.gpsimd.tensor_add(qbfdr[:, i, csl], tmp, qc)
                            elif sc == 0:
                                nc.gpsimd.tensor_add(kbfdr[:, i, 0:nfx],
                                                     tmp[:, 0:nfx], qc[:, 0:nfx])
                # partition shuffle DR layout -> head-pair [64|64] layout
                for pr in range(2):
                    for a in range(2):
                        hh = 2 * pr + a
                        for i in range(2):
                            po = slice(64 * a + 32 * i, 64 * a + 32 * i + 32)
                            pi = slice(32 * hh, 32 * hh + 32)
                            nc.gpsimd.dma_start(out=qbf[s][ht][pr][po, :], in_=qbfdr[pi, i, :])
                            nc.sync.dma_start(out=kbf[s][ht][pr][po, :], in_=kbfdr[pi, i, :])

            def v_load(s, prep_p):
                wvf = prep_p.tile([P, 8, NHL * D], bf16, tag=f"wvf{s}", name="wvf")
                nc.gpsimd.dma_start(out=wvf, in_=wv_d[s][:].rearrange("(c p) n -> p c n", p=P))
                for st in range(8):
                    nc.gpsimd.memset(v_sb[s][st][:, :, D:D + 1], 1.0)
                return wvf

            def v_fills(s, xh, wvf, halves=(0, 1)):
                for nh in halves:
                    nsl = slice(nh * 256, (nh + 1) * 256)
                    for st in range(8):
                        psv = mmp.tile([P, 512], f32, tag="mm", name="psv")
                        for kc in range(8):
                            nc.tensor.matmul(
                                psv[:, 0:256],
                                xh[kc][:, st * P:(st + 1) * P],
                                wvf[:, kc, nsl],
                                start=(kc == 0), stop=(kc == 7),
                            )
                        nc.vector.tensor_copy(
                            out=v_sb[s][st][:, 4 * nh:4 * nh + 4, 0:D],
                            in_=psv[:, 0:256].rearrange("p (h d) -> p h d", d=D),
                        )

            # ---------------- attention per head (all 16 key-chunks) ---------
            # spread the 4 fast-exp chunks through the head; lead with an
            # ACT chunk so the PV accumulation group never waits on DVE
            TST_ORDER = (2, 0, 3, 4, 5, 1, 6, 7)

            acc_live = {}

            def head_attn(s, ht, hh, ts_range=(0, 1)):
                h = 4 * ht + hh
                pr = slice(32 * hh, 32 * hh + 32)
                pair, a = hh // 2, hh % 2
                pp64 = slice(64 * a, 64 * a + 64)
                if 0 in ts_range:
                    accs = [accp.tile([P, 4, D + 1], f32, tag="accA", name="accA"),
                            accp.tile([P, 4, D + 1], f32, tag="accB", name="accB")]
                    acc_live[(s, h)] = accs
                else:
                    accs = acc_live[(s, h)]
                for cidx, (ts, tst) in enumerate(
                        (t, o) for t in range(2) for o in TST_ORDER):
                    if ts not in ts_range:
                        continue
                    first, last = cidx == 0, cidx == 15
                    sc_ps = scp.tile([P, S], f32, tag="sc", name="sc_ps")
                    if tst in FX_TST:
                        # bf16 scores + vector-engine fast-exp pun
                        with tc.high_priority(offset=-20):
                            for sc in range(2):
                                csl = slice(sc * 512, (sc + 1) * 512)
                                nc.tensor.matmul(
                                    sc_ps[:, csl],
                                    kbf[ts][ht][pair][pp64, tst * P:(tst + 1) * P],
                                    qbf[s][ht][pair][pp64, csl],
                                )
                        esx = esb_p.tile([P, S], i16, tag="esx", bufs=4, name="esx")
                        with tc.high_priority(offset=-40):
                            nc.vector.tensor_scalar(
                                out=esx, in0=sc_ps,
                                scalar1=A_FX, scalar2=B_FX,
                                op0=ALU.mult, op1=ALU.add,
                            )
                        es = esx.bitcast(bf16)
                    else:
                        with tc.high_priority(offset=-20):
                            for sc in range(2):
                                csl = slice(sc * 512, (sc + 1) * 512)
                                nc.tensor.matmul(
                                    sc_ps[:, csl],
                                    kdr[ts][ht][pr, :, tst * P:(tst + 1) * P],
                                    qdr[s][ht][pr, :, csl],
                                    perf_mode=DR,
                                    tile_position=(32 * hh, 0),
                                )
                        est = esb_p.tile([P, S], bf16, tag="es", bufs=9, name="es")
                        nc.scalar.activation(out=est, in_=sc_ps, func=AF.Exp,
                                             bias=zeroc, scale=SCALE)
                        es = est
                    with tc.high_priority(offset=-25):
                        for qch in range(8):
                            nc.tensor.matmul(
                                accs[qch // 4][:, qch % 4, :],
                                es[:, qch * P:(qch + 1) * P],
                                v_sb[ts][tst][:, h, :],
                                start=first, stop=last,
                            )
                if 1 not in ts_range:
                    return
                # normalize: per-q denominators sit on the free axis (col 64)
                for grp in range(2):
                    rr = small.tile([P, 4], f32, tag="rr", bufs=2, name="rr")
                    nc.vector.reciprocal(out=rr, in_=accs[grp][:, :, D])
                    for j in range(4):
                        qch = grp * 4 + j
                        nc.vector.tensor_scalar_mul(
                            attn2[s][qch][:, h * D:(h + 1) * D],
                            accs[grp][:, j, 0:D],
                            rr[:, j:j + 1],
                        )

            def transposes(s, ps):
                for p in ps:
                    for qch in range(8):
                        nc.sync.dma_start_transpose(
                            out=attnT[s][p][:, qch * P:(qch + 1) * P],
                            in_=attn2[s][qch][:, p * P:(p + 1) * P],
                        )

            # ---------------- out-projection ----------------
            opar = {}

            def tail_a(s, wop, wo_t):
                for p in range(2):
                    nc.sync.dma_start(out=wo_t[p], in_=wout_d[s][p * P:(p + 1) * P, :])
                for st in range(8):
                    for oc in range(2):
                        pso = mmp.tile([P, 512], f32, tag="mm", name="pso")
                        for p in range(2):
                            nc.tensor.matmul(
                                pso,
                                attnT[s][p][:, st * P:(st + 1) * P],
                                wo_t[p][:, oc * 512:(oc + 1) * 512],
                                start=(p == 0), stop=(p == 1),
                            )
                        op_t = wop.tile([P, 512], bf16, tag=f"opar{st}_{oc}", name="opar")
                        nc.vector.tensor_copy(out=op_t, in_=pso)
                        opar[(s, st, oc)] = op_t

            def tail_b(s, wop, wo_t):
                for p in range(2, 4):
                    nc.sync.dma_start(out=wo_t[p], in_=wout_d[s][p * P:(p + 1) * P, :])
                for st in range(8):
                    for oc in range(2):
                        pso = mmp.tile([P, 512], f32, tag="mm", name="pso")
                        for p in range(2, 4):
                            nc.tensor.matmul(
                                pso,
                                attnT[s][p][:, st * P:(st + 1) * P],
                                wo_t[p][:, oc * 512:(oc + 1) * 512],
                                start=(p == 2), stop=(p == 3),
                            )
                        osb = small.tile([P, 512], f32, tag="osb", bufs=3, name="osb")
                        nc.vector.tensor_add(osb, pso, opar[(s, st, oc)])
                        (nc.gpsimd if (st + oc) % 2 == 0 else nc.sync).dma_start(
                            out=out_d[s][st * P:(st + 1) * P, oc * 512:(oc + 1) * 512], in_=osb)

            # ---------------- emission ----------------
            with tc.tile_pool(name="prep", bufs=1) as prep_p:
                xh0, cq0 = load_stream(0, prep_p)
                qk_group(0, xh0, cq0, 0, prep_p)
                wvf0 = v_load(0, prep_p)
                with tc.high_priority(offset=-50):
                    v_fills(0, xh0, wvf0, halves=(0,))
                xh1, cq1 = load_stream(1, prep_p)
                head_attn(0, 0, 0, ts_range=(0,))
                qk_group(1, xh1, cq1, 0, prep_p)
                wvf1 = v_load(1, prep_p)
                v_fills(1, xh1, wvf1, halves=(0,))
                head_attn(0, 0, 0, ts_range=(1,))
                for hh in range(1, 4):
                    head_attn(0, 0, hh)
                qk_group(0, xh0, cq0, 1, prep_p)
                v_fills(0, xh0, wvf0, halves=(1,))
                for hh in range(4):
                    head_attn(1, 0, hh)
                qk_group(1, xh1, cq1, 1, prep_p)
                v_fills(1, xh1, wvf1, halves=(1,))
                for hh in range(4):
                    head_attn(0, 1, hh)
            with tc.tile_pool(name="wo", bufs=1) as wop:
                wo_t0 = [wop.tile([P, H], bf16, tag=f"wo{p}", name=f"wo{p}") for p in range(4)]
                wo_t1 = [wop.tile([P, H], bf16, tag=f"wo{p}", name=f"wo{p}") for p in range(4)]
                transposes(0, (0, 1, 2, 3))
                tail_a(0, wop, wo_t0)
                tail_b(0, wop, wo_t0)
                transposes(1, (0, 1))
                tail_a(1, wop, wo_t1)
                for hh in range(4):
                    head_attn(1, 1, hh)
                transposes(1, (2, 3))
                tail_b(1, wop, wo_t1)

    if do_compile:
        nc.compile()
    return nc


def _host_prep(x_a, x_b, Wqkv_a, Wqkv_b, Wout_a, Wout_b,
               gamma_a, beta_a, gamma_b, beta_b, height, width):
    """Build the 8 per-core input maps."""
    import ml_dtypes
    cos, sin = _rope_tables(height, width)      # [S, 64]

    # DR rope tables [128, 2, S]: partition p = 32*hh + dl (repeats over hh)
    dl = np.arange(32)
    cos_dr = np.empty((P, 2, S), np.float32)
    sin_dr = np.empty((P, 2, S), np.float32)
    for hh in range(4):
        rows = 32 * hh + dl
        cos_dr[rows, 0, :] = cos[:, dl].T          # cos[s, dl]
        cos_dr[rows, 1, :] = cos[:, 32 + dl].T
        sin_dr[rows, 0, :] = -sin[:, dl].T         # pre-negated for i=0
        sin_dr[rows, 1, :] = sin[:, 32 + dl].T
    cos_dr = np.ascontiguousarray(cos_dr.reshape(P, 2 * S).astype(ml_dtypes.bfloat16))
    sin_dr = np.ascontiguousarray(sin_dr.reshape(P, 2 * S).astype(ml_dtypes.bfloat16))

    # host LayerNorm fold: xhat = r*(x-mu), shipped transposed [H, S]
    def xhat(x):
        x = x.astype(np.float32)
        mu = x.mean(axis=-1, keepdims=True)
        var = ((x - mu) ** 2).mean(axis=-1, keepdims=True)
        r = 1.0 / np.sqrt(var + LN_EPS)
        return ((x - mu) * r).astype(np.float32)

    streams = []
    vshifts = []
    for (W, Wo, g, b) in ((Wqkv_a, Wout_a, gamma_a, beta_a), (Wqkv_b, Wout_b, gamma_b, beta_b)):
        Wg = (W * g[:, None]).astype(np.float32)       # gamma-folded
        cfull = (b.astype(np.float64) @ W.astype(np.float64)).astype(np.float32)  # beta@W [3H]
        W4 = Wg.reshape(H, 3, NH, D)
        c4 = cfull.reshape(3, NH, D)
        per_hg = []
        for hg in range(2):
            h0 = hg * NHL
            # DR column order: blocks (qk, ht, half) of 128 cols = (hh, dl)
            cols = []
            ccols = []
            for qk in range(2):
                for ht in range(2):
                    for half in range(2):
                        for hh in range(4):
                            head = h0 + 4 * ht + hh
                            dsl = slice(32 * half, 32 * half + 32)
                            cols.append(W4[:, qk, head, dsl])      # [H, 32]
                            ccols.append(c4[qk, head, dsl])        # [32]
            wqk = np.ascontiguousarray(np.concatenate(cols, axis=1).astype(ml_dtypes.bfloat16))
            # beta@W per qk column as per-partition scalars [128, 8 blocks]
            cqk_blk = np.ascontiguousarray(
                np.concatenate(ccols).reshape(8, P).T.astype(np.float32))   # [128, 8]
            wv = np.ascontiguousarray(W4[:, 2, h0:h0 + NHL, :].reshape(H, NHL * D).astype(ml_dtypes.bfloat16))
            wout = np.ascontiguousarray(
                Wo.reshape(NH, D, H)[h0:h0 + NHL].reshape(NHL * D, H).astype(ml_dtypes.bfloat16))
            per_hg.append(dict(wqk=wqk, wv=wv, cqk=cqk_blk, wout=wout))
        # exact host-side V correction: beta@Wv shifts attn uniformly
        # (softmax weights sum to 1), so it lands as a constant row on out
        vshift = (cfull[2 * H:3 * H].astype(np.float64) @ Wo.astype(np.float64)).astype(np.float32)
        streams.append(per_hg)
        vshifts.append(vshift)

    in_maps = []
    B = x_a.shape[0]
    xh_a = [np.ascontiguousarray(xhat(x_a[b_i]).T.astype(ml_dtypes.bfloat16)) for b_i in range(B)]
    xh_b = [np.ascontiguousarray(xhat(x_b[b_i]).T.astype(ml_dtypes.bfloat16)) for b_i in range(B)]
    for c in range(N_CORES):
        b_i, hg = (c // 2) % B, c % 2
        m = {
            "xh_s0": xh_a[b_i],
            "xh_s1": xh_b[b_i],
            "cosdr": cos_dr, "sindr": sin_dr,
        }
        for s in range(2):
            blk = streams[s][hg]
            m[f"wqk_s{s}"] = blk["wqk"]
            m[f"wv_s{s}"] = blk["wv"]
            m[f"cqk_s{s}"] = blk["cqk"]
            m[f"wout_s{s}"] = blk["wout"]
        in_maps.append(m)
    return in_maps, vshifts


def kernel(x_a, x_b, Wqkv_a, Wqkv_b, Wout_a, Wout_b,
           gamma_a, beta_a, gamma_b, beta_b, height, width):
    from concourse.bass_utils import run_bass_kernel_spmd

    x_a = np.asarray(x_a, dtype=np.float32)
    x_b = np.asarray(x_b, dtype=np.float32)
    B = x_a.shape[0]
    in_maps, vshifts = _host_prep(x_a, x_b,
                         np.asarray(Wqkv_a, np.float32), np.asarray(Wqkv_b, np.float32),
                         np.asarray(Wout_a, np.float32), np.asarray(Wout_b, np.float32),
                         np.asarray(gamma_a, np.float32), np.asarray(beta_a, np.float32),
                         np.asarray(gamma_b, np.float32), np.asarray(beta_b, np.float32),
                         height, width)
    nc = _get_program()
    res = run_bass_kernel_spmd(nc, in_maps, list(range(N_CORES))).results
    out_a = np.empty((B, S, H), np.float32)
    out_b = np.empty((B, S, H), np.float32)
    for b_i in range(B):
        out_a[b_i] = res[2 * b_i]["out_s0"] + res[2 * b_i + 1]["out_s0"] + vshifts[0]
        out_b[b_i] = res[2 * b_i]["out_s1"] + res[2 * b_i + 1]["out_s1"] + vshifts[1]
    return out_a, out_b


def _get_program():
    global _PROGRAM
    if _PROGRAM is None:
        _PROGRAM = _build_program()
    return _PROGRAM
